# revision 1
# baseline (speedup 1.0000x reference)
"""Root-to-leaves TreeLSTM over a complete binary tree (depth 17, 131071 nodes,
feat=h=512), distributed over 8 TRN2 NeuronCores with zero inter-core
communication.

Sharding: level d's nodes split into 8 contiguous chunks means each core's
chunk at level d+1 is exactly the children of its chunk at level d, so each
core owns one of the 8 subtrees rooted at level 3. Levels 0-3 are replicated
on all cores; because the program is SPMD (one NEFF for all cores), each
core's copy of the replicated levels is relabeled by XOR with the core index
prefix so that "my subtree root" sits at column 0 for every core with a
position-independent parent map (parent of col k is col k//2 always).

Device layout: everything is transposed host-side — features arrive as
[512(feat), cols] bf16 so the feat/h contraction dim sits on SBUF partitions
and no on-device transposes are needed. Within each level the columns are
permuted to [left-children | right-children] so the parent h-state GEMM
operand and the parent c-state vector are contiguous slices (no broadcasts).

Per level: one fused GEMM [iofux_w; px_w]^T (24 M-tiles of 128) over the
features plus iofuh_w^T over the stored parent h (bf16), accumulated in the
same PSUM tile; Sigmoid/Tanh/Identity+bias applied by ScalarE directly from
PSUM; the c/h elementwise chain runs fp32 on VectorE/GpSimd; c state kept
fp32 in SBUF, h state bf16 in SBUF (it is only ever a bf16 GEMM input).
"""

import os
import sys

sys.path.insert(0, "/opt/trn_rl_repo")

import numpy as np
import ml_dtypes
from contextlib import ExitStack

import concourse.bass as bass
import concourse.mybir as mybir
import concourse.tile as tile
from concourse import bacc

P = 128
KT = 4              # 512 / 128 contraction tiles
H = 512
F = 512
DEPTH = 17
NCORES = 8
CHUNK = 512
M_IOFU = 20         # iofu M-tiles (2560/128)
M_ALL = 24          # + px M-tiles (512/128)
SPLIT_THRESH = 2048  # split last-2 levels when parent level exceeds this
BF16 = mybir.dt.bfloat16
F32 = mybir.dt.float32
AF = mybir.ActivationFunctionType
np_bf16 = ml_dtypes.bfloat16


def _level_sizes(depth):
    # per-core column count per level: levels 0..3 replicated, >=4 core-private
    return [1 << d if d <= 3 else 1 << (d - 3) for d in range(depth)]


def _plan(depth):
    """Segment schedule. Each seg = (level, seg_start, seg_len) in within-level
    logical coords. The last level's parent level is split in halves when it
    would otherwise need >2048 state columns, interleaving the two subtrees to
    halve peak state SBUF."""
    Ns = _level_sizes(depth)
    off = [0]
    for n in Ns:
        off.append(off[-1] + n)
    segs = []
    split = depth >= 2 and Ns[-2] > SPLIT_THRESH
    if split:
        for d in range(depth - 2):
            segs.append((d, 0, Ns[d]))
        for h in range(2):
            segs.append((depth - 2, h * Ns[depth - 2] // 2, Ns[depth - 2] // 2))
            segs.append((depth - 1, h * Ns[depth - 1] // 2, Ns[depth - 1] // 2))
    else:
        segs = [(d, 0, Ns[d]) for d in range(depth)]
    stored = [s for s in segs if s[0] < depth - 1]
    store_cols = max(s[2] for s in stored) if stored else 1
    return Ns, off, segs, split, store_cols


def build_nc(depth=DEPTH):
    """Build the SPMD single-core Bass program (same NEFF for all 8 cores)."""
    Ns, off, segs, split, store_cols = _plan(depth)
    C = off[-1]

    nc = bacc.Bacc("TRN2", target_bir_lowering=False, debug=False)
    featsT = nc.declare_dram_parameter("featsT", [F, C], BF16, isOutput=False)
    wxT = nc.declare_dram_parameter("wxT", [F, M_ALL * P], BF16, isOutput=False)
    whT = nc.declare_dram_parameter("whT", [H, M_IOFU * P], BF16, isOutput=False)
    biasm = nc.declare_dram_parameter("biasm", [P, M_ALL], F32, isOutput=False)
    ident = nc.declare_dram_parameter("ident", [P, P], BF16, isOutput=False)
    outT = nc.declare_dram_parameter("outT", [H, C], F32, isOutput=True)

    featsT_r = featsT[:].rearrange("(a p) c -> p a c", p=P)
    wxT_r = wxT[:].rearrange("(a p) m -> p a m", p=P)
    whT_r = whT[:].rearrange("(a p) m -> p a m", p=P)
    outT_r = outT[:].rearrange("(a p) c -> p a c", p=P)

    with ExitStack() as ctx:
        tc = ctx.enter_context(tile.TileContext(nc))
        wpool = ctx.enter_context(tc.tile_pool(name="w", bufs=1))
        spool = ctx.enter_context(tc.tile_pool(name="state", bufs=1))
        fpool = ctx.enter_context(tc.tile_pool(name="feats", bufs=3))
        pspool = ctx.enter_context(tc.tile_pool(name="ps", bufs=8, space="PSUM"))
        gpool = ctx.enter_context(tc.tile_pool(name="gates", bufs=10))
        xpool = ctx.enter_context(tc.tile_pool(name="px", bufs=4))
        tpool = ctx.enter_context(tc.tile_pool(name="tmp", bufs=7))
        opool = ctx.enter_context(tc.tile_pool(name="hf", bufs=3))
        ghpool = ctx.enter_context(tc.tile_pool(name="gh", bufs=5))

        wx_sb = wpool.tile([P, KT, M_ALL * P], BF16, tag="wx")
        wh_sb = wpool.tile([P, KT, M_IOFU * P], BF16, tag="wh")
        bias_sb = wpool.tile([P, M_ALL], F32, tag="bias")
        id_sb = wpool.tile([P, P], BF16, tag="ident")
        nc.sync.dma_start(wx_sb[:], wxT_r)
        nc.sync.dma_start(wh_sb[:], whT_r)
        nc.sync.dma_start(bias_sb[:], biasm[:])
        nc.sync.dma_start(id_sb[:], ident[:])

        # state double buffers: c fp32, hf bf16 (level d -> buffer d % 2)
        cst = [spool.tile([P, KT, store_cols], F32, tag=f"c{b}", name=f"c{b}")
               for b in (0, 1)]
        hst = [spool.tile([P, KT, store_cols], BF16, tag=f"h{b}", name=f"h{b}")
               for b in (0, 1)]

        def v3(ap):
            # flat [P, w] chunk view -> [P, 2, w//2] b-major (left|right block)
            return ap.rearrange("p (b q) -> p b q", b=2)

        def chunk_merged(d, col0, p0, w, store, buf):
            """Both L/R passes of a full small level (w = N <= 512) in one
            chunk: x-GEMM over all w cols, h-GEMM rhs = parent h read twice
            via a 0-step broadcast AP, state writes via b-major strided APs."""
            pbuf = (d - 1) % 2
            half = w // 2
            ft = fpool.tile([P, KT, CHUNK], BF16, tag="feats")
            nc.sync.dma_start(ft[:, :, :w], featsT_r[:, :, col0:col0 + w])
            for t in range(KT):
                ps_px = pspool.tile([P, CHUNK], F32, tag="ps")
                for k in range(KT):
                    m = M_IOFU + t
                    nc.tensor.matmul(
                        ps_px[:, :w], wx_sb[:, k, m * P:(m + 1) * P], ft[:, k, :w],
                        start=(k == 0), stop=(k == KT - 1))
                px = xpool.tile([P, CHUNK], F32, tag="px")
                nc.scalar.activation(px[:, :w], ps_px[:, :w], AF.Identity,
                                     bias=bias_sb[:, M_IOFU + t:M_IOFU + t + 1])
                gates = []
                for gi in range(5):
                    m = gi * KT + t
                    ps = pspool.tile([P, CHUNK], F32, tag="ps")
                    for k in range(KT):
                        nc.tensor.matmul(
                            ps[:, :w], wx_sb[:, k, m * P:(m + 1) * P], ft[:, k, :w],
                            start=(k == 0), stop=False)
                    for k in range(KT):
                        hb = hst[pbuf][:, k, None, p0:p0 + half].to_broadcast(
                            (P, 2, half))
                        nc.tensor.matmul(
                            ps[:, :w], wh_sb[:, k, m * P:(m + 1) * P], hb,
                            start=False, stop=(k == KT - 1))
                    g = gpool.tile([P, CHUNK], F32, tag="gates")
                    func = AF.Tanh if gi == 3 else AF.Sigmoid
                    nc.scalar.activation(g[:, :w], ps[:, :w], func,
                                         bias=bias_sb[:, m:m + 1])
                    gates.append(g)
                gi_, go_, gf_, gu_, gr_ = gates

                if store:
                    c_dst = cst[buf][:, t, 0:w].rearrange("p (q b) -> p b q", b=2)
                else:
                    c_dst = v3(tpool.tile([P, CHUNK], F32, tag="tmp",
                                          name="ctmp2")[:, :w])
                pc_b = cst[pbuf][:, t, None, p0:p0 + half].to_broadcast(
                    (P, 2, half))
                t1 = tpool.tile([P, CHUNK], F32, tag="tmp")
                t2 = tpool.tile([P, CHUNK], F32, tag="tmp")
                nc.vector.tensor_mul(t1[:, :w], gi_[:, :w], gu_[:, :w])
                nc.vector.tensor_mul(v3(t2[:, :w]), v3(gf_[:, :w]), pc_b)
                nc.vector.tensor_add(c_dst, v3(t1[:, :w]), v3(t2[:, :w]))
                tc_ = tpool.tile([P, CHUNK], F32, tag="tmp")
                nc.scalar.activation(v3(tc_[:, :w]), c_dst, AF.Tanh)
                t3 = tpool.tile([P, CHUNK], F32, tag="tmp")
                nc.vector.tensor_mul(t3[:, :w], go_[:, :w], tc_[:, :w])
                d_ = tpool.tile([P, CHUNK], F32, tag="tmp")
                nc.gpsimd.tensor_sub(d_[:, :w], t3[:, :w], px[:, :w])
                e_ = tpool.tile([P, CHUNK], F32, tag="tmp")
                nc.gpsimd.tensor_mul(e_[:, :w], gr_[:, :w], d_[:, :w])
                hf = opool.tile([P, CHUNK], F32, tag="hf")
                nc.vector.tensor_add(hf[:, :w], e_[:, :w], px[:, :w])
                nc.sync.dma_start(outT_r[:, t, col0:col0 + w], hf[:, :w])
                if store:
                    nc.gpsimd.tensor_copy(
                        hst[buf][:, t, 0:w].rearrange("p (q b) -> p b q", b=2),
                        v3(hf[:, :w]))

        def elemwise(t, w, gates, px, pc_ap, c_dst, h_dst, col0):
            gi_, go_, gf_, gu_, gr_ = gates
            t1 = tpool.tile([P, CHUNK], F32, tag="tmp")
            t2 = tpool.tile([P, CHUNK], F32, tag="tmp")
            nc.vector.tensor_mul(t1[:, :w], gi_[:, :w], gu_[:, :w])
            nc.vector.tensor_mul(t2[:, :w], gf_[:, :w], pc_ap)
            nc.vector.tensor_add(c_dst, t1[:, :w], t2[:, :w])
            tc_ = tpool.tile([P, CHUNK], F32, tag="tmp")
            nc.scalar.activation(tc_[:, :w], c_dst, AF.Tanh)
            t3 = tpool.tile([P, CHUNK], F32, tag="tmp")
            nc.vector.tensor_mul(t3[:, :w], go_[:, :w], tc_[:, :w])
            d_ = tpool.tile([P, CHUNK], F32, tag="tmp")
            nc.gpsimd.tensor_sub(d_[:, :w], t3[:, :w], px[:, :w])
            e_ = tpool.tile([P, CHUNK], F32, tag="tmp")
            nc.gpsimd.tensor_mul(e_[:, :w], gr_[:, :w], d_[:, :w])
            hf = opool.tile([P, CHUNK], F32, tag="hf")
            nc.vector.tensor_add(hf[:, :w], e_[:, :w], px[:, :w])
            nc.sync.dma_start(outT_r[:, t, col0:col0 + w], hf[:, :w])
            if h_dst is not None:
                nc.gpsimd.tensor_copy(h_dst, hf[:, :w])

        def chunk_pair(d, colL, colR, p0, w, store, buf, wq0):
            """L and R passes for parents [p0, p0+w): the parent h-GEMM runs
            once per gate tile (into its own PSUM, copied to SBUF bf16) and is
            added into both children's PSUM via identity-matmul accumulation —
            saving 1/4 of the h-GEMM matmul cycles."""
            pbuf = (d - 1) % 2
            ftL = fpool.tile([P, KT, CHUNK], BF16, tag="feats")
            ftR = fpool.tile([P, KT, CHUNK], BF16, tag="feats")
            nc.sync.dma_start(ftL[:, :, :w], featsT_r[:, :, colL:colL + w])
            nc.sync.dma_start(ftR[:, :, :w], featsT_r[:, :, colR:colR + w])
            for t in range(KT):
                pxs = []
                for ft in (ftL, ftR):
                    ps_px = pspool.tile([P, CHUNK], F32, tag="ps")
                    for k in range(KT):
                        m = M_IOFU + t
                        nc.tensor.matmul(
                            ps_px[:, :w], wx_sb[:, k, m * P:(m + 1) * P],
                            ft[:, k, :w], start=(k == 0), stop=(k == KT - 1))
                    px = xpool.tile([P, CHUNK], F32, tag="px")
                    nc.scalar.activation(px[:, :w], ps_px[:, :w], AF.Identity,
                                         bias=bias_sb[:, M_IOFU + t:M_IOFU + t + 1])
                    pxs.append(px)
                ghs = []
                for gi in range(5):
                    m = gi * KT + t
                    ps_h = pspool.tile([P, CHUNK], F32, tag="ps")
                    for k in range(KT):
                        nc.tensor.matmul(
                            ps_h[:, :w], wh_sb[:, k, m * P:(m + 1) * P],
                            hst[pbuf][:, k, p0:p0 + w],
                            start=(k == 0), stop=(k == KT - 1))
                    gh = ghpool.tile([P, CHUNK], BF16, tag="gh")
                    nc.vector.tensor_copy(gh[:, :w], ps_h[:, :w])
                    ghs.append(gh)
                gatesL, gatesR = [], []
                for gi in range(5):
                    m = gi * KT + t
                    for ft, gl in ((ftL, gatesL), (ftR, gatesR)):
                        ps = pspool.tile([P, CHUNK], F32, tag="ps")
                        for k in range(KT):
                            nc.tensor.matmul(
                                ps[:, :w], wx_sb[:, k, m * P:(m + 1) * P],
                                ft[:, k, :w], start=(k == 0), stop=False)
                        nc.tensor.matmul(ps[:, :w], id_sb[:], ghs[gi][:, :w],
                                         start=False, stop=True)
                        g = gpool.tile([P, CHUNK], F32, tag="gates")
                        func = AF.Tanh if gi == 3 else AF.Sigmoid
                        nc.scalar.activation(g[:, :w], ps[:, :w], func,
                                             bias=bias_sb[:, m:m + 1])
                        gl.append(g)
                pc_ap = cst[pbuf][:, t, p0:p0 + w]
                for b, gates, px, col0 in ((0, gatesL, pxs[0], colL),
                                           (1, gatesR, pxs[1], colR)):
                    if store:
                        c_dst = cst[buf][:, t, 2 * wq0 + b: 2 * (wq0 + w) + b - 1: 2]
                        h_dst = hst[buf][:, t, 2 * wq0 + b: 2 * (wq0 + w) + b - 1: 2]
                    else:
                        c_dst = tpool.tile([P, CHUNK], F32, tag="tmp",
                                           name="ctmp3")[:, :w]
                        h_dst = None
                    elemwise(t, w, gates, px, pc_ap, c_dst, h_dst, col0)

        def chunk(d, col0, p0, w, store, buf, wq0, b):
            """One chunk of w node-columns at level d.
            col0: featsT/outT column base; p0: parent position in parent state
            buffers (ignored for d == 0); store: write c/h state; buf: this
            level's state buffer idx; wq0: within-pass col offset for state
            writes; b: 0 = left-children pass, 1 = right."""
            pbuf = (d - 1) % 2
            ft = fpool.tile([P, KT, CHUNK], BF16, tag="feats")
            nc.sync.dma_start(ft[:, :, :w], featsT_r[:, :, col0:col0 + w])
            for t in range(KT):
                # px tile: M-tile 20+t
                ps_px = pspool.tile([P, CHUNK], F32, tag="ps")
                for k in range(KT):
                    m = M_IOFU + t
                    nc.tensor.matmul(
                        ps_px[:, :w], wx_sb[:, k, m * P:(m + 1) * P], ft[:, k, :w],
                        start=(k == 0), stop=(k == KT - 1))
                px = xpool.tile([P, CHUNK], F32, tag="px")
                nc.scalar.activation(px[:, :w], ps_px[:, :w], AF.Identity,
                                     bias=bias_sb[:, M_IOFU + t:M_IOFU + t + 1])
                gates = []
                for gi in range(5):  # i, o, f, u, r
                    m = gi * KT + t
                    ps = pspool.tile([P, CHUNK], F32, tag="ps")
                    for k in range(KT):
                        nc.tensor.matmul(
                            ps[:, :w], wx_sb[:, k, m * P:(m + 1) * P], ft[:, k, :w],
                            start=(k == 0), stop=(k == KT - 1 and d == 0))
                    if d > 0:
                        for k in range(KT):
                            nc.tensor.matmul(
                                ps[:, :w], wh_sb[:, k, m * P:(m + 1) * P],
                                hst[pbuf][:, k, p0:p0 + w],
                                start=False, stop=(k == KT - 1))
                    g = gpool.tile([P, CHUNK], F32, tag="gates")
                    func = AF.Tanh if gi == 3 else AF.Sigmoid
                    nc.scalar.activation(g[:, :w], ps[:, :w], func,
                                         bias=bias_sb[:, m:m + 1])
                    gates.append(g)
                gi_, go_, gf_, gu_, gr_ = gates

                # c = i*u + f*pc   (written straight into state, stride 2)
                if store:
                    c_dst = cst[buf][:, t, 2 * wq0 + b: 2 * (wq0 + w) + b - 1: 2]
                else:
                    c_dst = tpool.tile([P, CHUNK], F32, tag="tmp", name="ctmp")[:, :w]
                if d > 0:
                    t1 = tpool.tile([P, CHUNK], F32, tag="tmp")
                    t2 = tpool.tile([P, CHUNK], F32, tag="tmp")
                    nc.vector.tensor_mul(t1[:, :w], gi_[:, :w], gu_[:, :w])
                    nc.vector.tensor_mul(t2[:, :w], gf_[:, :w],
                                         cst[pbuf][:, t, p0:p0 + w])
                    nc.vector.tensor_add(c_dst, t1[:, :w], t2[:, :w])
                else:
                    nc.vector.tensor_mul(c_dst, gi_[:, :w], gu_[:, :w])
                # h = o * tanh(c); hf = px + r*(h - px)
                tc_ = tpool.tile([P, CHUNK], F32, tag="tmp")
                nc.scalar.activation(tc_[:, :w], c_dst, AF.Tanh)
                t3 = tpool.tile([P, CHUNK], F32, tag="tmp")
                nc.vector.tensor_mul(t3[:, :w], go_[:, :w], tc_[:, :w])
                d_ = tpool.tile([P, CHUNK], F32, tag="tmp")
                nc.gpsimd.tensor_sub(d_[:, :w], t3[:, :w], px[:, :w])
                e_ = tpool.tile([P, CHUNK], F32, tag="tmp")
                nc.gpsimd.tensor_mul(e_[:, :w], gr_[:, :w], d_[:, :w])
                hf = opool.tile([P, CHUNK], F32, tag="hf")
                nc.vector.tensor_add(hf[:, :w], e_[:, :w], px[:, :w])
                nc.sync.dma_start(outT_r[:, t, col0:col0 + w], hf[:, :w])
                if store:
                    nc.gpsimd.tensor_copy(
                        hst[buf][:, t, 2 * wq0 + b: 2 * (wq0 + w) + b - 1: 2],
                        hf[:, :w])

        for (d, s, l) in segs:
            store = d < depth - 1
            buf = d % 2
            parent_base = s // 2 if (d == depth - 1 and split) else 0
            if d == 0:
                chunk(0, off[0], 0, 1, store, buf, 0, 0)
                continue
            if l == Ns[d] and l <= CHUNK:
                chunk_merged(d, off[d], 0, l, store, buf)
                continue
            plen = l // 2
            for q0 in range(0, plen, CHUNK):
                w = min(CHUNK, plen - q0)
                colL = off[d] + s // 2 + q0
                colR = off[d] + Ns[d] // 2 + s // 2 + q0
                p0 = s // 2 + q0 - parent_base
                chunk_pair(d, colL, colR, p0, w, store, buf, q0)

    nc.compile()
    return nc, C


# ---------------------------------------------------------------- host side

def _col_maps(depth):
    """Per (core, level): global node indices for each comp-order column."""
    Ns, off, _, _, _ = _level_sizes(depth), None, None, None, None
    Ns = _level_sizes(depth)
    maps = []  # maps[core][level] -> np.int64 [N_d] global node idx per column
    for i in range(NCORES):
        per_level = []
        for d in range(depth):
            N = Ns[d]
            logical = np.concatenate([np.arange(0, N, 2), np.arange(1, N, 2)])
            if d <= 3:
                orig = logical ^ (i >> (3 - d))
            else:
                orig = i * (1 << (d - 3)) + logical
            per_level.append(((1 << d) - 1) + orig)
        maps.append(per_level)
    return maps


def prep_inputs(features, px_w, px_b, iofux_w, iofux_b, iofuh_w, iofuh_b,
                depth=DEPTH):
    Ns = _level_sizes(depth)
    C = sum(Ns)
    features = np.asarray(features, np.float32)
    wx = np.concatenate([np.asarray(iofux_w, np.float32),
                         np.asarray(px_w, np.float32)], axis=0)  # [3072, 512]
    wxT = np.ascontiguousarray(wx.T).astype(np_bf16)             # [512, 3072]
    whT = np.ascontiguousarray(np.asarray(iofuh_w, np.float32).T).astype(np_bf16)
    bias_all = np.concatenate([
        np.asarray(iofux_b, np.float32) + np.asarray(iofuh_b, np.float32),
        np.asarray(px_b, np.float32)])                           # [3072]
    biasm = np.ascontiguousarray(bias_all.reshape(M_ALL, P).T)   # [128, 24]

    maps = _col_maps(depth)
    idm = np.eye(P, dtype=np_bf16)
    in_maps = []
    for i in range(NCORES):
        cols = np.concatenate(maps[i])                           # [C]
        fcore = features[cols, :]                                # [C, 512] f32
        fT = np.ascontiguousarray(fcore.T).astype(np_bf16)       # [512, C]
        in_maps.append({"featsT": fT, "wxT": wxT, "whT": whT,
                        "biasm": biasm, "ident": idm})
    return in_maps, maps, C


def assemble_output(results, maps, depth=DEPTH):
    Ns = _level_sizes(depth)
    n_nodes = (1 << depth) - 1
    out = np.empty((n_nodes, H), np.float32)
    offs = np.cumsum([0] + Ns)
    for i in range(NCORES):
        o = results[i]["outT"]                                   # [512, C] f32
        for d in range(depth):
            if d <= 3 and i != 0:
                continue  # replicated levels: take core 0's copy
            cols = maps[i][d]
            out[cols, :] = o[:, offs[d]:offs[d + 1]].T
    return out


_CACHE = {}


def _get_built(depth=DEPTH):
    if depth not in _CACHE:
        _CACHE[depth] = build_nc(depth)
    return _CACHE[depth]


def run_cores(in_maps, depth=DEPTH, trace=False):
    from concourse.bass_utils import run_bass_kernel_spmd
    nc, C = _get_built(depth)
    br = run_bass_kernel_spmd(nc, in_maps, list(range(NCORES)), trace=trace)
    return br


def kernel(features, px_w, px_b, iofux_w, iofux_b, iofuh_w, iofuh_b):
    in_maps, maps, C = prep_inputs(features, px_w, px_b, iofux_w, iofux_b,
                                   iofuh_w, iofuh_b)
    br = run_cores(in_maps)
    return assemble_output(br.results, maps)



# revision 3
# speedup vs baseline: 1.2040x; 1.2040x over previous
"""Root-to-leaves TreeLSTM over a complete binary tree (depth 17, 131071 nodes,
feat=h=512), distributed over 8 TRN2 NeuronCores with zero inter-core
communication.

Sharding: level d's nodes split into 8 contiguous chunks means each core's
chunk at level d+1 is exactly the children of its chunk at level d, so each
core owns one of the 8 subtrees rooted at level 3. Levels 0-3 are replicated
on all cores; the SPMD program relabels them per-core by XOR with the core
index prefix so "my subtree root" is column 0 everywhere and the parent map
is position-independent (parent of col j is col j//2 in plain node order).

v2: fp8 DoubleRow matmuls for the iofu x-GEMM and h-GEMM (2x PE throughput),
both accumulated into the same 2-bank PSUM pair per gate; the parent-h moving
operand uses a stride-0 repeat AP so each parent feeds both of its children
in one GEMM (no copies / identity injections / L-R block permutation). The
px GEMM stays bf16 for accuracy (its error passes to the output unattenuated
by any sigmoid). Weights are pre-scaled into fp8's normal range (wx*64,
wh*16, h-state*4) and compensated by the activation scale (1/64). Gates, c
state, tmps and the output are bf16 (2x DVE, half DMA); h state is fp8
(DoubleRow input). Elementwise is split DVE/Pool to balance engines.
"""

import os
import sys

sys.path.insert(0, "/opt/trn_rl_repo")

import numpy as np
import ml_dtypes
from contextlib import ExitStack

import concourse.bass as bass
import concourse.mybir as mybir
import concourse.tile as tile
from concourse import bacc

P = 128
KT = 4               # 512 / 128 contraction tiles
H = 512
F = 512
DEPTH = 17
NCORES = 8
CHUNK = 1024         # children columns per chunk
M_IOFU = 20          # iofu M-tiles (2560/128), fp8
M_PX = 4             # px M-tiles (512/128), bf16
SPLIT_THRESH = 2048  # split last-2 levels when parent level exceeds this
WSCALE = 64.0        # wx fp8 pre-scale
WHSCALE = 16.0       # wh fp8 pre-scale
HSCALE = 4.0         # h-state fp8 pre-scale (WHSCALE*HSCALE == WSCALE)
BF16 = mybir.dt.bfloat16
FP8 = mybir.dt.float8e4
F32 = mybir.dt.float32
AF = mybir.ActivationFunctionType
DR = mybir.MatmulPerfMode.DoubleRow
np_bf16 = ml_dtypes.bfloat16
np_fp8 = ml_dtypes.float8_e4m3


def _level_sizes(depth):
    # per-core column count per level: levels 0..3 replicated, >=4 core-private
    return [1 << d if d <= 3 else 1 << (d - 3) for d in range(depth)]


def _plan(depth):
    """Segment schedule: seg = (level, seg_start, seg_len) in within-level
    child coords. The last level's parent level is split in halves when it
    would otherwise need > SPLIT_THRESH state columns."""
    Ns = _level_sizes(depth)
    off = [0]
    for n in Ns:
        off.append(off[-1] + n)
    segs = []
    split = depth >= 2 and Ns[-2] > SPLIT_THRESH
    if split:
        for d in range(depth - 2):
            segs.append((d, 0, Ns[d]))
        for h in range(2):
            segs.append((depth - 2, h * Ns[depth - 2] // 2, Ns[depth - 2] // 2))
            segs.append((depth - 1, h * Ns[depth - 1] // 2, Ns[depth - 1] // 2))
    else:
        segs = [(d, 0, Ns[d]) for d in range(depth)]
    stored = [s for s in segs if s[0] < depth - 1]
    store_cols = max(s[2] for s in stored) if stored else 1
    return Ns, off, segs, split, store_cols


def build_nc(depth=DEPTH):
    """Build the SPMD single-core Bass program (same NEFF for all 8 cores)."""
    Ns, off, segs, split, store_cols = _plan(depth)
    C = off[-1]

    nc = bacc.Bacc("TRN2", target_bir_lowering=False, debug=False)
    featsq = nc.declare_dram_parameter("featsq", [F, C], FP8, isOutput=False)
    featsb = nc.declare_dram_parameter("featsb", [F, C], BF16, isOutput=False)
    wxq = nc.declare_dram_parameter("wxq", [F, M_IOFU * P], FP8, isOutput=False)
    wxpx = nc.declare_dram_parameter("wxpx", [F, M_PX * P], BF16, isOutput=False)
    whq = nc.declare_dram_parameter("whq", [H, M_IOFU * P], FP8, isOutput=False)
    biasm = nc.declare_dram_parameter("biasm", [P, M_IOFU], F32, isOutput=False)
    pxb = nc.declare_dram_parameter("pxb", [P, M_PX], F32, isOutput=False)
    outT = nc.declare_dram_parameter("outT", [H, C], BF16, isOutput=True)

    featsq_r = featsq[:].rearrange("(a p) c -> p a c", p=P)
    featsb_r = featsb[:].rearrange("(a p) c -> p a c", p=P)
    wxq_r = wxq[:].rearrange("(a p) m -> p a m", p=P)
    wxpx_r = wxpx[:].rearrange("(a p) m -> p a m", p=P)
    whq_r = whq[:].rearrange("(a p) m -> p a m", p=P)
    outT_r = outT[:].rearrange("(a p) c -> p a c", p=P)

    with ExitStack() as ctx:
        tc = ctx.enter_context(tile.TileContext(nc))
        wpool = ctx.enter_context(tc.tile_pool(name="w", bufs=1))
        spool = ctx.enter_context(tc.tile_pool(name="state", bufs=1))
        fpool = ctx.enter_context(tc.tile_pool(name="feats", bufs=3))
        pspool = ctx.enter_context(tc.tile_pool(name="ps", bufs=4, space="PSUM"))
        gpool = ctx.enter_context(tc.tile_pool(name="gates", bufs=10))
        xpool = ctx.enter_context(tc.tile_pool(name="px", bufs=4))
        tpool = ctx.enter_context(tc.tile_pool(name="tmp", bufs=8))
        opool = ctx.enter_context(tc.tile_pool(name="hf", bufs=4))

        wx_sb = wpool.tile([P, KT, M_IOFU * P], FP8, tag="wxq")
        wpx_sb = wpool.tile([P, KT, M_PX * P], BF16, tag="wxpx")
        wh_sb = wpool.tile([P, KT, M_IOFU * P], FP8, tag="whq")
        bias_sb = wpool.tile([P, M_IOFU], F32, tag="biasm")
        pxb_sb = wpool.tile([P, M_PX], F32, tag="pxb")
        nc.sync.dma_start(wx_sb[:], wxq_r)
        nc.sync.dma_start(wpx_sb[:], wxpx_r)
        nc.sync.dma_start(wh_sb[:], whq_r)
        nc.sync.dma_start(bias_sb[:], biasm[:])
        nc.sync.dma_start(pxb_sb[:], pxb[:])

        # state double buffers (level d -> buffer d % 2): c bf16, h fp8 (*4)
        cst = [spool.tile([P, KT, store_cols], BF16, tag=f"c{b}", name=f"c{b}")
               for b in (0, 1)]
        hst = [spool.tile([P, KT, store_cols], FP8, tag=f"h{b}", name=f"h{b}")
               for b in (0, 1)]

        def chunk(d, col0, st0, pp0, w, store, buf):
            """One chunk of w child columns at level d (node order).
            col0: feats/outT column base; st0: state store offset; pp0:
            parent offset in the parent level's state buffers."""
            pbuf = (d - 1) % 2
            hw = w // 2  # parents in this chunk
            ftq = fpool.tile([P, KT, CHUNK], FP8, tag="ftq")
            ftb = fpool.tile([P, KT, CHUNK], BF16, tag="ftb")
            nc.sync.dma_start(ftq[:, :, :w], featsq_r[:, :, col0:col0 + w])
            nc.sync.dma_start(ftb[:, :, :w], featsb_r[:, :, col0:col0 + w])
            for t in range(KT):
                # ---- px GEMM (bf16) + evacuation with bias --------------
                ps_px = pspool.tile([P, CHUNK], F32, tag="ps")
                for k in range(KT):
                    for c0 in range(0, w, 512):
                        cw = min(512, w - c0)
                        nc.tensor.matmul(
                            ps_px[:, c0:c0 + cw],
                            wpx_sb[:, k, t * P:(t + 1) * P],
                            ftb[:, k, c0:c0 + cw],
                            start=(k == 0), stop=(k == KT - 1))
                px = xpool.tile([P, CHUNK], BF16, tag="px")
                nc.scalar.activation(px[:, :w], ps_px[:, :w], AF.Identity,
                                     bias=pxb_sb[:, t:t + 1])

                # ---- 5 gate GEMMs (fp8 DoubleRow, x + h fused) ----------
                gates = []
                for gi in range(5):
                    m = gi * KT + t
                    ps = pspool.tile([P, CHUNK], F32, tag="ps")
                    for kp in (0, 2):
                        for c0 in range(0, w, 256):
                            cw = min(256, w - c0)
                            last_x = (d == 0 and kp == 2
                                      and (c0 + 256 >= w or c0 + 256 == 512))
                            nc.tensor.matmul(
                                ps[:, c0:c0 + cw],
                                wx_sb[:, kp:kp + 2, m * P:(m + 1) * P],
                                ftq[:, kp:kp + 2, c0:c0 + cw],
                                perf_mode=DR,
                                start=(kp == 0 and c0 % 512 == 0),
                                stop=last_x)
                    if d > 0:
                        for kp in (0, 2):
                            for p0 in range(0, hw, P):
                                pw = min(P, hw - p0)
                                hrep = hst[pbuf][
                                    :, kp:kp + 2, pp0 + p0:pp0 + p0 + pw,
                                    None].to_broadcast((P, 2, pw, 2))
                                c0 = 2 * p0
                                last_h = (kp == 2
                                          and (c0 + 256 >= w or c0 + 256 == 512))
                                nc.tensor.matmul(
                                    ps[:, c0:c0 + 2 * pw],
                                    wh_sb[:, kp:kp + 2, m * P:(m + 1) * P],
                                    hrep, perf_mode=DR,
                                    start=False, stop=last_h)
                    g = gpool.tile([P, CHUNK], BF16, tag="gates")
                    func = AF.Tanh if gi == 3 else AF.Sigmoid
                    nc.scalar.activation(g[:, :w], ps[:, :w], func,
                                         bias=bias_sb[:, m:m + 1],
                                         scale=1.0 / WSCALE)
                    gates.append(g)
                gi_, go_, gf_, gu_, gr_ = gates

                # ---- elementwise (bf16), split DVE / Pool ---------------
                if store:
                    c_dst = cst[buf][:, t, st0:st0 + w]
                else:
                    c_dst = tpool.tile([P, CHUNK], BF16, tag="tmp",
                                       name="ctmp")[:, :w]
                if d > 0:
                    t1 = tpool.tile([P, CHUNK], BF16, tag="tmp")
                    t2 = tpool.tile([P, CHUNK], BF16, tag="tmp")
                    nc.vector.tensor_mul(t1[:, :w], gi_[:, :w], gu_[:, :w])
                    pc_rep = cst[pbuf][:, t, pp0:pp0 + hw, None].to_broadcast(
                        (P, hw, 2))
                    nc.gpsimd.tensor_mul(
                        t2[:, :w].rearrange("p (a b) -> p a b", b=2),
                        gf_[:, :w].rearrange("p (a b) -> p a b", b=2),
                        pc_rep)
                    nc.vector.tensor_add(c_dst, t1[:, :w], t2[:, :w])
                else:
                    nc.vector.tensor_mul(c_dst, gi_[:, :w], gu_[:, :w])
                tc_ = tpool.tile([P, CHUNK], BF16, tag="tmp")
                nc.scalar.activation(tc_[:, :w], c_dst, AF.Tanh)
                t3 = tpool.tile([P, CHUNK], BF16, tag="tmp")
                nc.vector.tensor_mul(t3[:, :w], go_[:, :w], tc_[:, :w])
                d_ = tpool.tile([P, CHUNK], BF16, tag="tmp")
                nc.vector.tensor_sub(d_[:, :w], t3[:, :w], px[:, :w])
                e_ = tpool.tile([P, CHUNK], BF16, tag="tmp")
                nc.gpsimd.tensor_mul(e_[:, :w], gr_[:, :w], d_[:, :w])
                hf = opool.tile([P, CHUNK], BF16, tag="hf")
                nc.vector.tensor_add(hf[:, :w], e_[:, :w], px[:, :w])
                nc.sync.dma_start(outT_r[:, t, col0:col0 + w], hf[:, :w])
                if store:
                    nc.gpsimd.tensor_scalar_mul(
                        hst[buf][:, t, st0:st0 + w], hf[:, :w], HSCALE)

        for (d, s, l) in segs:
            store = d < depth - 1
            buf = d % 2
            pbase = s // 2 if (split and d == depth - 1) else 0
            for q0 in range(s, s + l, CHUNK):
                w = min(CHUNK, s + l - q0)
                chunk(d, off[d] + q0, q0 - s, q0 // 2 - pbase, w, store, buf)

    nc.compile()
    return nc, C


# ---------------------------------------------------------------- host side

def _col_maps(depth):
    """Per (core, level): global node index for each column (node order)."""
    Ns = _level_sizes(depth)
    maps = []
    for i in range(NCORES):
        per_level = []
        for d in range(depth):
            base = (1 << d) - 1
            if d <= 3:
                orig = np.arange(Ns[d]) ^ (i >> (3 - d))
            else:
                orig = i * (1 << (d - 3)) + np.arange(Ns[d])
            per_level.append(base + orig)
        maps.append(per_level)
    return maps


def prep_inputs(features, px_w, px_b, iofux_w, iofux_b, iofuh_w, iofuh_b,
                depth=DEPTH):
    Ns = _level_sizes(depth)
    C = sum(Ns)
    features = np.asarray(features, np.float32)
    wxq = np.ascontiguousarray(
        (np.asarray(iofux_w, np.float32) * WSCALE).T).astype(np_fp8)
    wxpx = np.ascontiguousarray(
        np.asarray(px_w, np.float32).T).astype(np_bf16)
    whq = np.ascontiguousarray(
        (np.asarray(iofuh_w, np.float32) * WHSCALE).T).astype(np_fp8)
    bias_all = np.asarray(iofux_b, np.float32) + np.asarray(iofuh_b, np.float32)
    biasm = np.ascontiguousarray(bias_all.reshape(M_IOFU, P).T)
    pxbm = np.ascontiguousarray(
        np.asarray(px_b, np.float32).reshape(M_PX, P).T)

    maps = _col_maps(depth)
    in_maps = []
    for i in range(NCORES):
        cols = np.concatenate(maps[i])
        fcore = features[cols, :]                       # [C, 512] f32
        fT = np.ascontiguousarray(fcore.T)              # [512, C]
        in_maps.append({"featsq": fT.astype(np_fp8),
                        "featsb": fT.astype(np_bf16),
                        "wxq": wxq, "wxpx": wxpx, "whq": whq,
                        "biasm": biasm, "pxb": pxbm})
    return in_maps, maps, C


def assemble_output(results, maps, depth=DEPTH):
    Ns = _level_sizes(depth)
    n_nodes = (1 << depth) - 1
    out = np.empty((n_nodes, H), np.float32)
    offs = np.cumsum([0] + Ns)
    for i in range(NCORES):
        o = results[i]["outT"]                          # [512, C] bf16
        for d in range(depth):
            if d <= 3 and i != 0:
                continue  # replicated levels: take core 0's copy
            cols = maps[i][d]
            out[cols, :] = o[:, offs[d]:offs[d + 1]].T.astype(np.float32)
    return out


_CACHE = {}


def _get_built(depth=DEPTH):
    if depth not in _CACHE:
        _CACHE[depth] = build_nc(depth)
    return _CACHE[depth]


def run_cores(in_maps, depth=DEPTH, trace=False):
    from concourse.bass_utils import run_bass_kernel_spmd
    nc, C = _get_built(depth)
    br = run_bass_kernel_spmd(nc, in_maps, list(range(NCORES)), trace=trace)
    return br


def kernel(features, px_w, px_b, iofux_w, iofux_b, iofuh_w, iofuh_b):
    in_maps, maps, C = prep_inputs(features, px_w, px_b, iofux_w, iofux_b,
                                   iofuh_w, iofuh_b)
    br = run_cores(in_maps)
    return assemble_output(br.results, maps)


# revision 5
# speedup vs baseline: 1.3261x; 1.1014x over previous
"""Root-to-leaves TreeLSTM over a complete binary tree (depth 17, 131071 nodes,
feat=h=512), distributed over 8 TRN2 NeuronCores with zero inter-core
communication.

Sharding: level d's nodes split into 8 contiguous chunks means each core's
chunk at level d+1 is exactly the children of its chunk at level d, so each
core owns one of the 8 subtrees rooted at level 3. Levels 0-3 are replicated
on all cores; the SPMD program relabels them per-core by XOR with the core
index prefix so "my subtree root" is column 0 everywhere and the parent map
is position-independent (parent of col j is col j//2 in plain node order).

v2: fp8 DoubleRow matmuls for the iofu x-GEMM and h-GEMM (2x PE throughput),
both accumulated into the same 2-bank PSUM pair per gate; the parent-h moving
operand uses a stride-0 repeat AP so each parent feeds both of its children
in one GEMM (no copies / identity injections / L-R block permutation). The
px GEMM stays bf16 for accuracy (its error passes to the output unattenuated
by any sigmoid). Weights are pre-scaled into fp8's normal range (wx*64,
wh*16, h-state*4) and compensated by the activation scale (1/64). Gates, c
state, tmps and the output are bf16 (2x DVE, half DMA); h state is fp8
(DoubleRow input). Elementwise is split DVE/Pool to balance engines.
"""

import os
import sys

sys.path.insert(0, "/opt/trn_rl_repo")

import numpy as np
import ml_dtypes
from contextlib import ExitStack

import concourse.bass as bass
import concourse.mybir as mybir
import concourse.tile as tile
from concourse import bacc

P = 128
KT = 4               # 512 / 128 contraction tiles
H = 512
F = 512
DEPTH = 17
NCORES = 8
CHUNK = 1024         # children columns per chunk
M_IOFU = 20          # iofu M-tiles (2560/128), fp8
M_PX = 4             # px M-tiles (512/128), bf16
SPLIT_THRESH = 2048  # split last-2 levels when parent level exceeds this
WSCALE = 64.0        # wx fp8 pre-scale
WHSCALE = 16.0       # wh fp8 pre-scale
HSCALE = 4.0         # h-state fp8 pre-scale (WHSCALE*HSCALE == WSCALE)
BF16 = mybir.dt.bfloat16
FP8 = mybir.dt.float8e4
F32 = mybir.dt.float32
AF = mybir.ActivationFunctionType
DR = mybir.MatmulPerfMode.DoubleRow
np_bf16 = ml_dtypes.bfloat16
np_fp8 = ml_dtypes.float8_e4m3


def _level_sizes(depth):
    # per-core column count per level: levels 0..3 replicated, >=4 core-private
    return [1 << d if d <= 3 else 1 << (d - 3) for d in range(depth)]


def _plan(depth):
    """Segment schedule: seg = (level, seg_start, seg_len) in within-level
    child coords. The last level's parent level is split in halves when it
    would otherwise need > SPLIT_THRESH state columns."""
    Ns = _level_sizes(depth)
    off = [0]
    for n in Ns:
        off.append(off[-1] + n)
    segs = []
    split = depth >= 2 and Ns[-2] > SPLIT_THRESH
    if split:
        for d in range(depth - 2):
            segs.append((d, 0, Ns[d]))
        for h in range(2):
            segs.append((depth - 2, h * Ns[depth - 2] // 2, Ns[depth - 2] // 2))
            segs.append((depth - 1, h * Ns[depth - 1] // 2, Ns[depth - 1] // 2))
    else:
        segs = [(d, 0, Ns[d]) for d in range(depth)]
    stored = [s for s in segs if s[0] < depth - 1]
    store_cols = max(s[2] for s in stored) if stored else 1
    return Ns, off, segs, split, store_cols


def build_nc(depth=DEPTH):
    """Build the SPMD single-core Bass program (same NEFF for all 8 cores)."""
    Ns, off, segs, split, store_cols = _plan(depth)
    C = off[-1]

    nc = bacc.Bacc("TRN2", target_bir_lowering=False, debug=False)
    featsq = nc.declare_dram_parameter("featsq", [F, C], FP8, isOutput=False)
    featsb = nc.declare_dram_parameter("featsb", [F, C], BF16, isOutput=False)
    wxq = nc.declare_dram_parameter("wxq", [F, M_IOFU * P], FP8, isOutput=False)
    wxpx = nc.declare_dram_parameter("wxpx", [F, M_PX * P], BF16, isOutput=False)
    whq = nc.declare_dram_parameter("whq", [H, M_IOFU * P], FP8, isOutput=False)
    biasm = nc.declare_dram_parameter("biasm", [P, M_IOFU], F32, isOutput=False)
    pxb = nc.declare_dram_parameter("pxb", [P, M_PX], F32, isOutput=False)
    outT = nc.declare_dram_parameter("outT", [H, C], BF16, isOutput=True)

    featsq_r = featsq[:].rearrange("(a p) c -> p a c", p=P)
    featsb_r = featsb[:].rearrange("(a p) c -> p a c", p=P)
    wxq_r = wxq[:].rearrange("(a p) m -> p a m", p=P)
    wxpx_r = wxpx[:].rearrange("(a p) m -> p a m", p=P)
    whq_r = whq[:].rearrange("(a p) m -> p a m", p=P)
    outT_r = outT[:].rearrange("(a p) c -> p a c", p=P)

    with ExitStack() as ctx:
        tc = ctx.enter_context(tile.TileContext(nc))
        wpool = ctx.enter_context(tc.tile_pool(name="w", bufs=1))
        spool = ctx.enter_context(tc.tile_pool(name="state", bufs=1))
        fpool = ctx.enter_context(tc.tile_pool(name="feats", bufs=3))
        pspool = ctx.enter_context(tc.tile_pool(name="ps", bufs=4, space="PSUM"))
        gpool = ctx.enter_context(tc.tile_pool(name="gates", bufs=10))
        xpool = ctx.enter_context(tc.tile_pool(name="px", bufs=4))
        tpool = ctx.enter_context(tc.tile_pool(name="tmp", bufs=8))
        opool = ctx.enter_context(tc.tile_pool(name="hf", bufs=4))

        wx_sb = wpool.tile([P, KT, M_IOFU * P], FP8, tag="wxq")
        wpx_sb = wpool.tile([P, KT, M_PX * P], BF16, tag="wxpx")
        wh_sb = wpool.tile([P, KT, M_IOFU * P], FP8, tag="whq")
        bias_sb = wpool.tile([P, M_IOFU], F32, tag="biasm")
        pxb_sb = wpool.tile([P, M_PX], F32, tag="pxb")
        nc.sync.dma_start(wx_sb[:], wxq_r)
        nc.sync.dma_start(wpx_sb[:], wxpx_r)
        nc.sync.dma_start(wh_sb[:], whq_r)
        nc.sync.dma_start(bias_sb[:], biasm[:])
        nc.sync.dma_start(pxb_sb[:], pxb[:])

        # state double buffers (level d -> buffer d % 2): c bf16, h fp8 (*4)
        cst = [spool.tile([P, KT, store_cols], BF16, tag=f"c{b}", name=f"c{b}")
               for b in (0, 1)]
        hst = [spool.tile([P, KT, store_cols], FP8, tag=f"h{b}", name=f"h{b}")
               for b in (0, 1)]

        def chunk(d, col0, st0, pp0, w, store, buf):
            """One chunk of w child columns at level d (node order).
            col0: feats/outT column base; st0: state store offset; pp0:
            parent offset in the parent level's state buffers."""
            pbuf = (d - 1) % 2
            hw = w // 2  # parents in this chunk
            ftq = fpool.tile([P, KT, CHUNK], FP8, tag="ftq")
            ftb = fpool.tile([P, KT, CHUNK], BF16, tag="ftb")
            nc.sync.dma_start(ftq[:, :, :w], featsq_r[:, :, col0:col0 + w])
            nc.sync.dma_start(ftb[:, :, :w], featsb_r[:, :, col0:col0 + w])
            for t in range(KT):
                # ---- px GEMM (bf16) + evacuation with bias --------------
                ps_px = pspool.tile([P, CHUNK], F32, tag="ps")
                for k in range(KT):
                    for c0 in range(0, w, 512):
                        cw = min(512, w - c0)
                        nc.tensor.matmul(
                            ps_px[:, c0:c0 + cw],
                            wpx_sb[:, k, t * P:(t + 1) * P],
                            ftb[:, k, c0:c0 + cw],
                            start=(k == 0), stop=(k == KT - 1))
                px = xpool.tile([P, CHUNK], BF16, tag="px")
                nc.scalar.activation(px[:, :w], ps_px[:, :w], AF.Identity,
                                     bias=pxb_sb[:, t:t + 1])

                # ---- 5 gate GEMMs (fp8 DoubleRow, x + h fused) ----------
                gates = []
                for gi in range(5):
                    m = gi * KT + t
                    ps = pspool.tile([P, CHUNK], F32, tag="ps")
                    for kp in (0, 2):
                        for c0 in range(0, w, 256):
                            cw = min(256, w - c0)
                            last_x = (d == 0 and kp == 2
                                      and (c0 + 256 >= w or c0 + 256 == 512))
                            nc.tensor.matmul(
                                ps[:, c0:c0 + cw],
                                wx_sb[:, kp:kp + 2, m * P:(m + 1) * P],
                                ftq[:, kp:kp + 2, c0:c0 + cw],
                                perf_mode=DR,
                                start=(kp == 0 and c0 % 512 == 0),
                                stop=last_x)
                    if d > 0:
                        for kp in (0, 2):
                            for p0 in range(0, hw, P):
                                pw = min(P, hw - p0)
                                hrep = hst[pbuf][
                                    :, kp:kp + 2, pp0 + p0:pp0 + p0 + pw,
                                    None].to_broadcast((P, 2, pw, 2))
                                c0 = 2 * p0
                                last_h = (kp == 2
                                          and (c0 + 256 >= w or c0 + 256 == 512))
                                nc.tensor.matmul(
                                    ps[:, c0:c0 + 2 * pw],
                                    wh_sb[:, kp:kp + 2, m * P:(m + 1) * P],
                                    hrep, perf_mode=DR,
                                    start=False, stop=last_h)
                    g = gpool.tile([P, CHUNK], BF16, tag="gates")
                    func = AF.Tanh if gi == 3 else AF.Sigmoid
                    nc.scalar.activation(g[:, :w], ps[:, :w], func,
                                         bias=bias_sb[:, m:m + 1],
                                         scale=1.0 / WSCALE)
                    gates.append(g)
                gi_, go_, gf_, gu_, gr_ = gates

                # ---- elementwise (bf16), split DVE / Pool ---------------
                if store:
                    c_dst = cst[buf][:, t, st0:st0 + w]
                else:
                    c_dst = tpool.tile([P, CHUNK], BF16, tag="tmp",
                                       name="ctmp")[:, :w]
                if d > 0:
                    t1 = tpool.tile([P, CHUNK], BF16, tag="tmp")
                    t2 = tpool.tile([P, CHUNK], BF16, tag="tmp")
                    nc.vector.tensor_mul(t1[:, :w], gi_[:, :w], gu_[:, :w])
                    pc_rep = cst[pbuf][:, t, pp0:pp0 + hw, None].to_broadcast(
                        (P, hw, 2))
                    nc.vector.tensor_mul(
                        t2[:, :w].rearrange("p (a b) -> p a b", b=2),
                        gf_[:, :w].rearrange("p (a b) -> p a b", b=2),
                        pc_rep)
                    nc.vector.tensor_add(c_dst, t1[:, :w], t2[:, :w])
                else:
                    nc.vector.tensor_mul(c_dst, gi_[:, :w], gu_[:, :w])
                tc_ = tpool.tile([P, CHUNK], BF16, tag="tmp")
                nc.scalar.activation(tc_[:, :w], c_dst, AF.Tanh)
                t3 = tpool.tile([P, CHUNK], BF16, tag="tmp")
                nc.vector.tensor_mul(t3[:, :w], go_[:, :w], tc_[:, :w])
                d_ = tpool.tile([P, CHUNK], BF16, tag="tmp")
                nc.vector.tensor_sub(d_[:, :w], t3[:, :w], px[:, :w])
                e_ = tpool.tile([P, CHUNK], BF16, tag="tmp")
                nc.vector.tensor_mul(e_[:, :w], gr_[:, :w], d_[:, :w])
                hf = opool.tile([P, CHUNK], BF16, tag="hf")
                nc.vector.tensor_add(hf[:, :w], e_[:, :w], px[:, :w])
                if store:
                    nc.gpsimd.tensor_scalar_mul(
                        hst[buf][:, t, st0:st0 + w], hf[:, :w], HSCALE)
                nc.sync.dma_start(outT_r[:, t, col0:col0 + w], hf[:, :w])

        for (d, s, l) in segs:
            store = d < depth - 1
            buf = d % 2
            pbase = s // 2 if (split and d == depth - 1) else 0
            for q0 in range(s, s + l, CHUNK):
                w = min(CHUNK, s + l - q0)
                chunk(d, off[d] + q0, q0 - s, q0 // 2 - pbase, w, store, buf)

    nc.compile()
    return nc, C


# ---------------------------------------------------------------- host side

def _col_maps(depth):
    """Per (core, level): global node index for each column (node order)."""
    Ns = _level_sizes(depth)
    maps = []
    for i in range(NCORES):
        per_level = []
        for d in range(depth):
            base = (1 << d) - 1
            if d <= 3:
                orig = np.arange(Ns[d]) ^ (i >> (3 - d))
            else:
                orig = i * (1 << (d - 3)) + np.arange(Ns[d])
            per_level.append(base + orig)
        maps.append(per_level)
    return maps


def prep_inputs(features, px_w, px_b, iofux_w, iofux_b, iofuh_w, iofuh_b,
                depth=DEPTH):
    Ns = _level_sizes(depth)
    C = sum(Ns)
    features = np.asarray(features, np.float32)
    wxq = np.ascontiguousarray(
        (np.asarray(iofux_w, np.float32) * WSCALE).T).astype(np_fp8)
    wxpx = np.ascontiguousarray(
        np.asarray(px_w, np.float32).T).astype(np_bf16)
    whq = np.ascontiguousarray(
        (np.asarray(iofuh_w, np.float32) * WHSCALE).T).astype(np_fp8)
    bias_all = np.asarray(iofux_b, np.float32) + np.asarray(iofuh_b, np.float32)
    biasm = np.ascontiguousarray(bias_all.reshape(M_IOFU, P).T)
    pxbm = np.ascontiguousarray(
        np.asarray(px_b, np.float32).reshape(M_PX, P).T)

    maps = _col_maps(depth)
    in_maps = []
    for i in range(NCORES):
        cols = np.concatenate(maps[i])
        fcore = features[cols, :]                       # [C, 512] f32
        fT = np.ascontiguousarray(fcore.T)              # [512, C]
        in_maps.append({"featsq": fT.astype(np_fp8),
                        "featsb": fT.astype(np_bf16),
                        "wxq": wxq, "wxpx": wxpx, "whq": whq,
                        "biasm": biasm, "pxb": pxbm})
    return in_maps, maps, C


def assemble_output(results, maps, depth=DEPTH):
    Ns = _level_sizes(depth)
    n_nodes = (1 << depth) - 1
    out = np.empty((n_nodes, H), np.float32)
    offs = np.cumsum([0] + Ns)
    for i in range(NCORES):
        o = results[i]["outT"]                          # [512, C] bf16
        for d in range(depth):
            if d <= 3 and i != 0:
                continue  # replicated levels: take core 0's copy
            cols = maps[i][d]
            out[cols, :] = o[:, offs[d]:offs[d + 1]].T.astype(np.float32)
    return out


_CACHE = {}


def _get_built(depth=DEPTH):
    if depth not in _CACHE:
        _CACHE[depth] = build_nc(depth)
    return _CACHE[depth]


def run_cores(in_maps, depth=DEPTH, trace=False):
    from concourse.bass_utils import run_bass_kernel_spmd
    nc, C = _get_built(depth)
    br = run_bass_kernel_spmd(nc, in_maps, list(range(NCORES)), trace=trace)
    return br


def kernel(features, px_w, px_b, iofux_w, iofux_b, iofuh_w, iofuh_b):
    in_maps, maps, C = prep_inputs(features, px_w, px_b, iofux_w, iofux_b,
                                   iofuh_w, iofuh_b)
    br = run_cores(in_maps)
    return assemble_output(br.results, maps)


# revision 7
# speedup vs baseline: 1.6722x; 1.2609x over previous
"""Root-to-leaves TreeLSTM over a complete binary tree (depth 17, 131071 nodes,
feat=h=512), distributed over 8 TRN2 NeuronCores with zero inter-core
communication.

Sharding: level d's nodes split into 8 contiguous chunks means each core's
chunk at level d+1 is exactly the children of its chunk at level d, so each
core owns one of the 8 subtrees rooted at level 3. Levels 0-3 are replicated
on all cores; the SPMD program relabels them per-core by XOR with the core
index prefix so "my subtree root" is column 0 everywhere and the parent map
is position-independent (parent of col j is col j//2 in plain node order).

v2: fp8 DoubleRow matmuls for the iofu x-GEMM and h-GEMM (2x PE throughput),
both accumulated into the same 2-bank PSUM pair per gate; the parent-h moving
operand uses a stride-0 repeat AP so each parent feeds both of its children
in one GEMM (no copies / identity injections / L-R block permutation). The
px GEMM stays bf16 for accuracy (its error passes to the output unattenuated
by any sigmoid). Weights are pre-scaled into fp8's normal range (wx*64,
wh*16, h-state*4) and compensated by the activation scale (1/64). Gates, c
state, tmps and the output are bf16 (2x DVE, half DMA); h state is fp8
(DoubleRow input). Elementwise is split DVE/Pool to balance engines.
"""

import os
import sys

sys.path.insert(0, "/opt/trn_rl_repo")

import numpy as np
import ml_dtypes
from contextlib import ExitStack

import concourse.bass as bass
import concourse.mybir as mybir
import concourse.tile as tile
from concourse import bacc

P = 128
KT = 4               # 512 / 128 contraction tiles
H = 512
F = 512
DEPTH = 17
NCORES = 8
CHUNK = 1024         # children columns per chunk
M_IOFU = 20          # iofu M-tiles (2560/128), fp8
M_PX = 4             # px M-tiles (512/128), bf16
SPLIT_THRESH = 2048  # split last-2 levels when parent level exceeds this
WSCALE = 64.0        # wx fp8 pre-scale
WHSCALE = 64.0       # wh fp8 pre-scale (h state stored unscaled)
TPACK_W = 256        # pack all 4 k-tiles into one psum pair when w <= this
BF16 = mybir.dt.bfloat16
FP8 = mybir.dt.float8e4
F32 = mybir.dt.float32
AF = mybir.ActivationFunctionType
DR = mybir.MatmulPerfMode.DoubleRow
np_bf16 = ml_dtypes.bfloat16
np_fp8 = ml_dtypes.float8_e4m3


def _level_sizes(depth):
    # per-core column count per level: levels 0..3 replicated, >=4 core-private
    return [1 << d if d <= 3 else 1 << (d - 3) for d in range(depth)]


def _plan(depth):
    """Segment schedule: seg = (level, seg_start, seg_len) in within-level
    child coords. The last level's parent level is split in halves when it
    would otherwise need > SPLIT_THRESH state columns."""
    Ns = _level_sizes(depth)
    off = [0]
    for n in Ns:
        off.append(off[-1] + n)
    segs = []
    split = depth >= 2 and Ns[-2] > SPLIT_THRESH
    if split:
        for d in range(depth - 2):
            segs.append((d, 0, Ns[d]))
        for h in range(2):
            segs.append((depth - 2, h * Ns[depth - 2] // 2, Ns[depth - 2] // 2))
            segs.append((depth - 1, h * Ns[depth - 1] // 2, Ns[depth - 1] // 2))
    else:
        segs = [(d, 0, Ns[d]) for d in range(depth)]
    stored = [s for s in segs if s[0] < depth - 1]
    store_cols = max(s[2] for s in stored) if stored else 1
    return Ns, off, segs, split, store_cols


def build_nc(depth=DEPTH):
    """Build the SPMD single-core Bass program (same NEFF for all 8 cores)."""
    Ns, off, segs, split, store_cols = _plan(depth)
    C = off[-1]

    nc = bacc.Bacc("TRN2", target_bir_lowering=False, debug=False)
    featsq = nc.declare_dram_parameter("featsq", [F, C], FP8, isOutput=False)
    featsb = nc.declare_dram_parameter("featsb", [F, C], BF16, isOutput=False)
    wxq = nc.declare_dram_parameter("wxq", [F, M_IOFU * P], FP8, isOutput=False)
    wxpx = nc.declare_dram_parameter("wxpx", [F, M_PX * P], BF16, isOutput=False)
    whq = nc.declare_dram_parameter("whq", [H, M_IOFU * P], FP8, isOutput=False)
    biasm = nc.declare_dram_parameter("biasm", [P, M_IOFU], F32, isOutput=False)
    pxb = nc.declare_dram_parameter("pxb", [P, M_PX], F32, isOutput=False)
    outT = nc.declare_dram_parameter("outT", [H, C], BF16, isOutput=True)

    featsq_r = featsq[:].rearrange("(a p) c -> p a c", p=P)
    featsb_r = featsb[:].rearrange("(a p) c -> p a c", p=P)
    wxq_r = wxq[:].rearrange("(a p) m -> p a m", p=P)
    wxpx_r = wxpx[:].rearrange("(a p) m -> p a m", p=P)
    whq_r = whq[:].rearrange("(a p) m -> p a m", p=P)
    outT_r = outT[:].rearrange("(a p) c -> p a c", p=P)

    with ExitStack() as ctx:
        tc = ctx.enter_context(tile.TileContext(nc))
        wpool = ctx.enter_context(tc.tile_pool(name="w", bufs=1))
        spool = ctx.enter_context(tc.tile_pool(name="state", bufs=1))
        fpool = ctx.enter_context(tc.tile_pool(name="feats", bufs=3))
        pspool = ctx.enter_context(tc.tile_pool(name="ps", bufs=4, space="PSUM"))
        gpool = ctx.enter_context(tc.tile_pool(name="gates", bufs=10))
        xpool = ctx.enter_context(tc.tile_pool(name="px", bufs=4))
        tpool = ctx.enter_context(tc.tile_pool(name="tmp", bufs=8))
        opool = ctx.enter_context(tc.tile_pool(name="hf", bufs=4))

        wx_sb = wpool.tile([P, KT, M_IOFU * P], FP8, tag="wxq")
        wpx_sb = wpool.tile([P, KT, M_PX * P], BF16, tag="wxpx")
        wh_sb = wpool.tile([P, KT, M_IOFU * P], FP8, tag="whq")
        bias_sb = wpool.tile([P, M_IOFU], F32, tag="biasm")
        pxb_sb = wpool.tile([P, M_PX], F32, tag="pxb")
        nc.sync.dma_start(wx_sb[:], wxq_r)
        nc.sync.dma_start(wpx_sb[:], wxpx_r)
        nc.sync.dma_start(wh_sb[:], whq_r)
        nc.sync.dma_start(bias_sb[:], biasm[:])
        nc.sync.dma_start(pxb_sb[:], pxb[:])

        # state double buffers (level d -> buffer d % 2): c bf16, h fp8 (*4)
        cst = [spool.tile([P, KT, store_cols], BF16, tag=f"c{b}", name=f"c{b}")
               for b in (0, 1)]
        hst = [spool.tile([P, KT, store_cols], FP8, tag=f"h{b}", name=f"h{b}")
               for b in (0, 1)]

        def elemwise(d, w, gates, px, pc_reps, c_dst, tv, h_dst, out_ap):
            """bf16 elementwise chain on [P, w]-flat tiles. tv(x) views a
            flat AP in the (possibly 3-dim) shape of c_dst/h_dst/out_ap;
            pc_reps = list of (flat-slice, repeat-AP) pairs for f*pc."""
            gi_, go_, gf_, gu_, gr_ = gates
            if d > 0:
                t1 = tpool.tile([P, CHUNK], BF16, tag="tmp")
                t2 = tpool.tile([P, CHUNK], BF16, tag="tmp")
                nc.vector.tensor_mul(t1[:, :w], gi_[:, :w], gu_[:, :w])
                for sl, pc_rep in pc_reps:
                    nc.gpsimd.tensor_mul(
                        t2[:, sl].rearrange("p (a b) -> p a b", b=2),
                        gf_[:, sl].rearrange("p (a b) -> p a b", b=2),
                        pc_rep)
                nc.vector.tensor_add(c_dst, tv(t1[:, :w]), tv(t2[:, :w]))
            else:
                nc.vector.tensor_mul(c_dst, tv(gi_[:, :w]), tv(gu_[:, :w]))
            tc_ = tpool.tile([P, CHUNK], BF16, tag="tmp")
            nc.scalar.activation(tv(tc_[:, :w]), c_dst, AF.Tanh)
            t3 = tpool.tile([P, CHUNK], BF16, tag="tmp")
            nc.vector.tensor_mul(t3[:, :w], go_[:, :w], tc_[:, :w])
            d_ = tpool.tile([P, CHUNK], BF16, tag="tmp")
            nc.gpsimd.tensor_sub(d_[:, :w], t3[:, :w], px[:, :w])
            e_ = tpool.tile([P, CHUNK], BF16, tag="tmp")
            nc.vector.tensor_mul(e_[:, :w], gr_[:, :w], d_[:, :w])
            hf = opool.tile([P, CHUNK], BF16, tag="hf")
            nc.vector.tensor_add(hf[:, :w], e_[:, :w], px[:, :w])
            if h_dst is not None:
                nc.gpsimd.tensor_copy(h_dst, tv(hf[:, :w]))
            nc.sync.dma_start(out_ap, tv(hf[:, :w]))

        def chunk_tpack(d, col0, st0, pp0, w, store, buf, ftq, ftb):
            """Small-level chunk (w <= TPACK_W): all 4 k-tiles packed into
            one psum pair per gate -> one activation per gate, 3-dim state
            APs, 4x fewer elementwise/activation ops."""
            pbuf = (d - 1) % 2
            hw = w // 2
            tw = KT * w
            tpb = max(1, 512 // w)  # t-slices per 2KB psum bank

            def first_t(t):
                return t % tpb == 0

            def last_t(t):
                return t % tpb == tpb - 1 or t == KT - 1

            ps_px = pspool.tile([P, CHUNK], F32, tag="ps")
            for t in range(KT):
                for k in range(KT):
                    nc.tensor.matmul(
                        ps_px[:, t * w:(t + 1) * w],
                        wpx_sb[:, k, t * P:(t + 1) * P], ftb[:, k, :w],
                        start=(first_t(t) and k == 0),
                        stop=(last_t(t) and k == KT - 1))
            px = xpool.tile([P, CHUNK], BF16, tag="px")
            nc.scalar.activation(px[:, :tw], ps_px[:, :tw], AF.Identity,
                                 bias=pxb_sb[:, 0:1])
            gates = []
            for gi in range(5):
                ps = pspool.tile([P, CHUNK], F32, tag="ps")
                for t in range(KT):
                    m = gi * KT + t
                    for kp in (0, 2):
                        nc.tensor.matmul(
                            ps[:, t * w:(t + 1) * w],
                            wx_sb[:, kp:kp + 2, m * P:(m + 1) * P],
                            ftq[:, kp:kp + 2, :w], perf_mode=DR,
                            start=(first_t(t) and kp == 0),
                            stop=(last_t(t) and kp == 2 and d == 0))
                    if d > 0:
                        for kp in (0, 2):
                            hrep = hst[pbuf][
                                :, kp:kp + 2, pp0:pp0 + hw,
                                None].to_broadcast((P, 2, hw, 2))
                            nc.tensor.matmul(
                                ps[:, t * w:t * w + 2 * hw],
                                wh_sb[:, kp:kp + 2, m * P:(m + 1) * P],
                                hrep, perf_mode=DR,
                                start=False, stop=(last_t(t) and kp == 2))
                g = gpool.tile([P, CHUNK], BF16, tag="gates")
                func = AF.Tanh if gi == 3 else AF.Sigmoid
                nc.scalar.activation(g[:, :tw], ps[:, :tw], func,
                                     bias=bias_sb[:, gi * KT:gi * KT + 1],
                                     scale=1.0 / WSCALE)
                gates.append(g)

            if store:
                c_dst = cst[buf][:, :, st0:st0 + w]
                h_dst = hst[buf][:, :, st0:st0 + w]
            else:
                c_dst = tpool.tile([P, CHUNK], BF16, tag="tmp",
                                   name="ctmp")[:, :tw].rearrange(
                                       "p (a b) -> p a b", a=KT)
                h_dst = None
            tv = lambda ap: ap.rearrange("p (a b) -> p a b", a=KT)
            pc_reps = []
            if d > 0:
                for t in range(KT):
                    pc_reps.append((
                        slice(t * w, (t + 1) * w),
                        cst[pbuf][:, t, pp0:pp0 + hw, None].to_broadcast(
                            (P, hw, 2))))
            elemwise(d, tw, gates, px, pc_reps, c_dst, tv, h_dst,
                     outT_r[:, :, col0:col0 + w])

        def chunk_big(d, col0, st0, pp0, w, store, buf, ftq, ftb):
            """Full-size chunk (w > TPACK_W): one psum pair per (gate, t)."""
            pbuf = (d - 1) % 2
            hw = w // 2
            for t in range(KT):
                ps_px = pspool.tile([P, CHUNK], F32, tag="ps")
                for k in range(KT):
                    for c0 in range(0, w, 512):
                        cw = min(512, w - c0)
                        nc.tensor.matmul(
                            ps_px[:, c0:c0 + cw],
                            wpx_sb[:, k, t * P:(t + 1) * P],
                            ftb[:, k, c0:c0 + cw],
                            start=(k == 0), stop=(k == KT - 1))
                px = xpool.tile([P, CHUNK], BF16, tag="px")
                nc.scalar.activation(px[:, :w], ps_px[:, :w], AF.Identity,
                                     bias=pxb_sb[:, t:t + 1])
                gates = []
                for gi in range(5):
                    m = gi * KT + t
                    ps = pspool.tile([P, CHUNK], F32, tag="ps")
                    for kp in (0, 2):
                        for c0 in range(0, w, 256):
                            cw = min(256, w - c0)
                            nc.tensor.matmul(
                                ps[:, c0:c0 + cw],
                                wx_sb[:, kp:kp + 2, m * P:(m + 1) * P],
                                ftq[:, kp:kp + 2, c0:c0 + cw],
                                perf_mode=DR,
                                start=(kp == 0 and c0 % 512 == 0),
                                stop=False)
                    for kp in (0, 2):
                        for p0 in range(0, hw, P):
                            pw = min(P, hw - p0)
                            hrep = hst[pbuf][
                                :, kp:kp + 2, pp0 + p0:pp0 + p0 + pw,
                                None].to_broadcast((P, 2, pw, 2))
                            c0 = 2 * p0
                            last_h = (kp == 2
                                      and (c0 + 256 >= w or c0 + 256 == 512))
                            nc.tensor.matmul(
                                ps[:, c0:c0 + 2 * pw],
                                wh_sb[:, kp:kp + 2, m * P:(m + 1) * P],
                                hrep, perf_mode=DR,
                                start=False, stop=last_h)
                    g = gpool.tile([P, CHUNK], BF16, tag="gates")
                    func = AF.Tanh if gi == 3 else AF.Sigmoid
                    nc.scalar.activation(g[:, :w], ps[:, :w], func,
                                         bias=bias_sb[:, m:m + 1],
                                         scale=1.0 / WSCALE)
                    gates.append(g)

                if store:
                    c_dst = cst[buf][:, t, st0:st0 + w]
                    h_dst = hst[buf][:, t, st0:st0 + w]
                else:
                    c_dst = tpool.tile([P, CHUNK], BF16, tag="tmp",
                                       name="ctmp")[:, :w]
                    h_dst = None
                tv = lambda ap: ap
                pc_reps = [(slice(0, w),
                            cst[pbuf][:, t, pp0:pp0 + hw, None].to_broadcast(
                                (P, hw, 2)))]
                elemwise(d, w, gates, px, pc_reps, c_dst, tv, h_dst,
                         outT_r[:, t, col0:col0 + w])

        def chunk(d, col0, st0, pp0, w, store, buf):
            """One chunk of w child columns at level d (node order).
            col0: feats/outT column base; st0: state store offset; pp0:
            parent offset in the parent level's state buffers."""
            ftq = fpool.tile([P, KT, CHUNK], FP8, tag="ftq")
            ftb = fpool.tile([P, KT, CHUNK], BF16, tag="ftb")
            nc.sync.dma_start(ftq[:, :, :w], featsq_r[:, :, col0:col0 + w])
            nc.sync.dma_start(ftb[:, :, :w], featsb_r[:, :, col0:col0 + w])
            if w <= TPACK_W:
                chunk_tpack(d, col0, st0, pp0, w, store, buf, ftq, ftb)
            else:
                chunk_big(d, col0, st0, pp0, w, store, buf, ftq, ftb)

        for (d, s, l) in segs:
            store = d < depth - 1
            buf = d % 2
            pbase = s // 2 if (split and d == depth - 1) else 0
            for q0 in range(s, s + l, CHUNK):
                w = min(CHUNK, s + l - q0)
                chunk(d, off[d] + q0, q0 - s, q0 // 2 - pbase, w, store, buf)

    nc.compile()
    return nc, C


# ---------------------------------------------------------------- host side

def _col_maps(depth):
    """Per (core, level): global node index for each column (node order)."""
    Ns = _level_sizes(depth)
    maps = []
    for i in range(NCORES):
        per_level = []
        for d in range(depth):
            base = (1 << d) - 1
            if d <= 3:
                orig = np.arange(Ns[d]) ^ (i >> (3 - d))
            else:
                orig = i * (1 << (d - 3)) + np.arange(Ns[d])
            per_level.append(base + orig)
        maps.append(per_level)
    return maps


def prep_inputs(features, px_w, px_b, iofux_w, iofux_b, iofuh_w, iofuh_b,
                depth=DEPTH):
    Ns = _level_sizes(depth)
    C = sum(Ns)
    features = np.asarray(features, np.float32)
    wxq = np.ascontiguousarray(
        (np.asarray(iofux_w, np.float32) * WSCALE).T).astype(np_fp8)
    wxpx = np.ascontiguousarray(
        np.asarray(px_w, np.float32).T).astype(np_bf16)
    whq = np.ascontiguousarray(
        (np.asarray(iofuh_w, np.float32) * WHSCALE).T).astype(np_fp8)
    bias_all = np.asarray(iofux_b, np.float32) + np.asarray(iofuh_b, np.float32)
    biasm = np.ascontiguousarray(bias_all.reshape(M_IOFU, P).T)
    pxbm = np.ascontiguousarray(
        np.asarray(px_b, np.float32).reshape(M_PX, P).T)

    maps = _col_maps(depth)
    in_maps = []
    for i in range(NCORES):
        cols = np.concatenate(maps[i])
        fcore = features[cols, :]                       # [C, 512] f32
        fT = np.ascontiguousarray(fcore.T)              # [512, C]
        in_maps.append({"featsq": fT.astype(np_fp8),
                        "featsb": fT.astype(np_bf16),
                        "wxq": wxq, "wxpx": wxpx, "whq": whq,
                        "biasm": biasm, "pxb": pxbm})
    return in_maps, maps, C


def assemble_output(results, maps, depth=DEPTH):
    Ns = _level_sizes(depth)
    n_nodes = (1 << depth) - 1
    out = np.empty((n_nodes, H), np.float32)
    offs = np.cumsum([0] + Ns)
    for i in range(NCORES):
        o = results[i]["outT"]                          # [512, C] bf16
        for d in range(depth):
            if d <= 3 and i != 0:
                continue  # replicated levels: take core 0's copy
            cols = maps[i][d]
            out[cols, :] = o[:, offs[d]:offs[d + 1]].T.astype(np.float32)
    return out


_CACHE = {}


def _get_built(depth=DEPTH):
    if depth not in _CACHE:
        _CACHE[depth] = build_nc(depth)
    return _CACHE[depth]


def run_cores(in_maps, depth=DEPTH, trace=False):
    from concourse.bass_utils import run_bass_kernel_spmd
    nc, C = _get_built(depth)
    br = run_bass_kernel_spmd(nc, in_maps, list(range(NCORES)), trace=trace)
    return br


def kernel(features, px_w, px_b, iofux_w, iofux_b, iofuh_w, iofuh_b):
    in_maps, maps, C = prep_inputs(features, px_w, px_b, iofux_w, iofux_b,
                                   iofuh_w, iofuh_b)
    br = run_cores(in_maps)
    return assemble_output(br.results, maps)


# revision 10
# speedup vs baseline: 1.8540x; 1.1087x over previous
"""Root-to-leaves TreeLSTM over a complete binary tree (depth 17, 131071 nodes,
feat=h=512), distributed over 8 TRN2 NeuronCores with zero inter-core
communication.

Sharding: level d's nodes split into 8 contiguous chunks means each core's
chunk at level d+1 is exactly the children of its chunk at level d, so each
core owns one of the 8 subtrees rooted at level 3. Levels 0-3 are replicated
on all cores; the SPMD program relabels them per-core by XOR with the core
index prefix so "my subtree root" is column 0 everywhere and the parent map
is position-independent (parent of col j is col j//2 in plain node order).

v2: fp8 DoubleRow matmuls for the iofu x-GEMM and h-GEMM (2x PE throughput),
both accumulated into the same 2-bank PSUM pair per gate; the parent-h moving
operand uses a stride-0 repeat AP so each parent feeds both of its children
in one GEMM (no copies / identity injections / L-R block permutation). The
px GEMM stays bf16 for accuracy (its error passes to the output unattenuated
by any sigmoid). Weights are pre-scaled into fp8's normal range (wx*64,
wh*16, h-state*4) and compensated by the activation scale (1/64). Gates, c
state, tmps and the output are bf16 (2x DVE, half DMA); h state is fp8
(DoubleRow input). Elementwise is split DVE/Pool to balance engines.
"""

import os
import sys

sys.path.insert(0, "/opt/trn_rl_repo")

import numpy as np
import ml_dtypes
from contextlib import ExitStack

import concourse.bass as bass
import concourse.mybir as mybir
import concourse.tile as tile
from concourse import bacc

P = 128
KT = 4               # 512 / 128 contraction tiles
H = 512
F = 512
DEPTH = 17
NCORES = 8
CHUNK = 1024         # children columns per chunk
M_IOFU = 20          # iofu M-tiles (2560/128), fp8
M_PX = 4             # px M-tiles (512/128), bf16
SPLIT_THRESH = 2048  # split last-2 levels when parent level exceeds this
WSCALE = 64.0        # wx fp8 pre-scale
WHSCALE = 64.0       # wh fp8 pre-scale (h state stored unscaled)
TPACK_W = 256        # pack all 4 k-tiles into one psum pair when w <= this
BF16 = mybir.dt.bfloat16
FP8 = mybir.dt.float8e4
F32 = mybir.dt.float32
AF = mybir.ActivationFunctionType
DR = mybir.MatmulPerfMode.DoubleRow
np_bf16 = ml_dtypes.bfloat16
np_fp8 = ml_dtypes.float8_e4m3


def _level_sizes(depth):
    # per-core column count per level: levels 0..3 replicated, >=4 core-private
    return [1 << d if d <= 3 else 1 << (d - 3) for d in range(depth)]


def _plan(depth):
    """Segment schedule: seg = (level, seg_start, seg_len) in within-level
    child coords. The last level's parent level is split in halves when it
    would otherwise need > SPLIT_THRESH state columns."""
    Ns = _level_sizes(depth)
    off = [0]
    for n in Ns:
        off.append(off[-1] + n)
    segs = []
    split = depth >= 2 and Ns[-2] > SPLIT_THRESH
    if split:
        for d in range(depth - 2):
            segs.append((d, 0, Ns[d]))
        for h in range(2):
            segs.append((depth - 2, h * Ns[depth - 2] // 2, Ns[depth - 2] // 2))
            segs.append((depth - 1, h * Ns[depth - 1] // 2, Ns[depth - 1] // 2))
    else:
        segs = [(d, 0, Ns[d]) for d in range(depth)]
    stored = [s for s in segs if s[0] < depth - 1]
    store_cols = max(s[2] for s in stored) if stored else 1
    return Ns, off, segs, split, store_cols


def build_nc(depth=DEPTH):
    """Build the SPMD single-core Bass program (same NEFF for all 8 cores)."""
    Ns, off, segs, split, store_cols = _plan(depth)
    C = off[-1]

    nc = bacc.Bacc("TRN2", target_bir_lowering=False, debug=False)
    featsq = nc.declare_dram_parameter("featsq", [F, C], FP8, isOutput=False)
    featsb = nc.declare_dram_parameter("featsb", [F, C], BF16, isOutput=False)
    wxq = nc.declare_dram_parameter("wxq", [F, M_IOFU * P], FP8, isOutput=False)
    wxpx = nc.declare_dram_parameter("wxpx", [F, M_PX * P], BF16, isOutput=False)
    whq = nc.declare_dram_parameter("whq", [H, M_IOFU * P], FP8, isOutput=False)
    biasm = nc.declare_dram_parameter("biasm", [P, M_IOFU], F32, isOutput=False)
    pxb = nc.declare_dram_parameter("pxb", [P, M_PX], F32, isOutput=False)
    outT = nc.declare_dram_parameter("outT", [H, C], BF16, isOutput=True)

    featsq_r = featsq[:].rearrange("(a p) c -> p a c", p=P)
    featsb_r = featsb[:].rearrange("(a p) c -> p a c", p=P)
    wxq_r = wxq[:].rearrange("(a p) m -> p a m", p=P)
    wxpx_r = wxpx[:].rearrange("(a p) m -> p a m", p=P)
    whq_r = whq[:].rearrange("(a p) m -> p a m", p=P)
    outT_r = outT[:].rearrange("(a p) c -> p a c", p=P)

    with ExitStack() as ctx:
        tc = ctx.enter_context(tile.TileContext(nc))
        wpool = ctx.enter_context(tc.tile_pool(name="w", bufs=1))
        spool = ctx.enter_context(tc.tile_pool(name="state", bufs=1))
        fpool = ctx.enter_context(tc.tile_pool(name="feats", bufs=3))
        pspool = ctx.enter_context(tc.tile_pool(name="ps", bufs=4, space="PSUM"))
        gpool = ctx.enter_context(tc.tile_pool(name="gates", bufs=10))
        xpool = ctx.enter_context(tc.tile_pool(name="px", bufs=4))
        tpool = ctx.enter_context(tc.tile_pool(name="tmp", bufs=8))
        opool = ctx.enter_context(tc.tile_pool(name="hf", bufs=4))

        wx_sb = wpool.tile([P, KT, M_IOFU * P], FP8, tag="wxq")
        wpx_sb = wpool.tile([P, KT, M_PX * P], BF16, tag="wxpx")
        wh_sb = wpool.tile([P, KT, M_IOFU * P], FP8, tag="whq")
        bias_sb = wpool.tile([P, M_IOFU], F32, tag="biasm")
        pxb_sb = wpool.tile([P, M_PX], F32, tag="pxb")
        nc.sync.dma_start(wx_sb[:], wxq_r)
        nc.sync.dma_start(wpx_sb[:], wxpx_r)
        nc.sync.dma_start(wh_sb[:], whq_r)
        nc.sync.dma_start(bias_sb[:], biasm[:])
        nc.sync.dma_start(pxb_sb[:], pxb[:])

        # state double buffers (level d -> buffer d % 2): c bf16, h fp8 (*4)
        cst = [spool.tile([P, KT, store_cols], BF16, tag=f"c{b}", name=f"c{b}")
               for b in (0, 1)]
        hst = [spool.tile([P, KT, store_cols], FP8, tag=f"h{b}", name=f"h{b}")
               for b in (0, 1)]

        def elemwise_early(d, w, gi_, gu_, gf_, px, pc_reps, c_dst, tv):
            """First part of the bf16 elementwise chain (after i/u/f acts):
            c = i*u + f*pc and tanh(c). Runs while o/r GEMMs proceed."""
            if d > 0:
                t1 = tpool.tile([P, CHUNK], BF16, tag="tmp")
                t2 = tpool.tile([P, CHUNK], BF16, tag="tmp")
                nc.vector.tensor_mul(t1[:, :w], gi_[:, :w], gu_[:, :w])
                for sl, pc_rep in pc_reps:
                    nc.gpsimd.tensor_mul(
                        t2[:, sl].rearrange("p (a b) -> p a b", b=2),
                        gf_[:, sl].rearrange("p (a b) -> p a b", b=2),
                        pc_rep)
                nc.vector.tensor_add(c_dst, tv(t1[:, :w]), tv(t2[:, :w]))
            else:
                nc.vector.tensor_mul(c_dst, tv(gi_[:, :w]), tv(gu_[:, :w]))
            tc_ = tpool.tile([P, CHUNK], BF16, tag="tmp")
            nc.scalar.activation(tv(tc_[:, :w]), c_dst, AF.Tanh)
            return tc_

        def elemwise_mid(w, go_, tc_, px):
            """After o's act: t3 = o*tanh(c); d = t3 - px."""
            t3 = tpool.tile([P, CHUNK], BF16, tag="tmp")
            nc.vector.tensor_mul(t3[:, :w], go_[:, :w], tc_[:, :w])
            d_ = tpool.tile([P, CHUNK], BF16, tag="tmp")
            nc.vector.tensor_sub(d_[:, :w], t3[:, :w], px[:, :w])
            return d_

        def elemwise_late(w, gr_, d_, px, tv, h_dst, out_ap):
            """After r's act (critical tail): e = r*d; hf = e + px; store."""
            e_ = tpool.tile([P, CHUNK], BF16, tag="tmp")
            nc.vector.tensor_mul(e_[:, :w], gr_[:, :w], d_[:, :w])
            hf = opool.tile([P, CHUNK], BF16, tag="hf")
            nc.vector.tensor_add(hf[:, :w], e_[:, :w], px[:, :w])
            if h_dst is not None:
                nc.vector.tensor_copy(h_dst, tv(hf[:, :w]))
            nc.sync.dma_start(out_ap, tv(hf[:, :w]))

        def chunk_tpack(d, col0, st0, pp0, w, store, buf, ftq, ftb):
            """Small-level chunk (w <= TPACK_W): all 4 k-tiles packed into
            one psum pair per gate -> one activation per gate, 3-dim state
            APs, 4x fewer elementwise/activation ops."""
            pbuf = (d - 1) % 2
            hw = w // 2
            tw = KT * w
            tpb = max(1, 512 // w)  # t-slices per 2KB psum bank

            def first_t(t):
                return t % tpb == 0

            def last_t(t):
                return t % tpb == tpb - 1 or t == KT - 1

            ps_px = pspool.tile([P, CHUNK], F32, tag="ps")
            for t in range(KT):
                for k in range(KT):
                    nc.tensor.matmul(
                        ps_px[:, t * w:(t + 1) * w],
                        wpx_sb[:, k, t * P:(t + 1) * P], ftb[:, k, :w],
                        start=(first_t(t) and k == 0),
                        stop=(last_t(t) and k == KT - 1))
            px = xpool.tile([P, CHUNK], BF16, tag="px")
            nc.scalar.activation(px[:, :tw], ps_px[:, :tw], AF.Identity,
                                 bias=pxb_sb[:, 0:1])

            def gate(gi):
                ps = pspool.tile([P, CHUNK], F32, tag="ps")
                for t in range(KT):
                    m = gi * KT + t
                    for kp in (0, 2):
                        nc.tensor.matmul(
                            ps[:, t * w:(t + 1) * w],
                            wx_sb[:, kp:kp + 2, m * P:(m + 1) * P],
                            ftq[:, kp:kp + 2, :w], perf_mode=DR,
                            start=(first_t(t) and kp == 0),
                            stop=(last_t(t) and kp == 2 and d == 0))
                    if d > 0:
                        for kp in (0, 2):
                            hrep = hst[pbuf][
                                :, kp:kp + 2, pp0:pp0 + hw,
                                None].to_broadcast((P, 2, hw, 2))
                            nc.tensor.matmul(
                                ps[:, t * w:t * w + 2 * hw],
                                wh_sb[:, kp:kp + 2, m * P:(m + 1) * P],
                                hrep, perf_mode=DR,
                                start=False, stop=(last_t(t) and kp == 2))
                g = gpool.tile([P, CHUNK], BF16, tag="gates")
                func = AF.Tanh if gi == 3 else AF.Sigmoid
                nc.scalar.activation(g[:, :tw], ps[:, :tw], func,
                                     bias=bias_sb[:, gi * KT:gi * KT + 1],
                                     scale=1.0 / WSCALE)
                return g

            if store:
                c_dst = cst[buf][:, :, st0:st0 + w]
                h_dst = hst[buf][:, :, st0:st0 + w]
            else:
                c_dst = tpool.tile([P, CHUNK], BF16, tag="tmp",
                                   name="ctmp")[:, :tw].rearrange(
                                       "p (a b) -> p a b", a=KT)
                h_dst = None
            tv = lambda ap: ap.rearrange("p (a b) -> p a b", a=KT)
            pc_reps = []
            if d > 0:
                for t in range(KT):
                    pc_reps.append((
                        slice(t * w, (t + 1) * w),
                        cst[pbuf][:, t, pp0:pp0 + hw, None].to_broadcast(
                            (P, hw, 2))))
            gi_ = gate(0)
            gu_ = gate(3)
            gf_ = gate(2)
            tc_ = elemwise_early(d, tw, gi_, gu_, gf_, px, pc_reps, c_dst, tv)
            go_ = gate(1)
            d_ = elemwise_mid(tw, go_, tc_, px)
            gr_ = gate(4)
            elemwise_late(tw, gr_, d_, px, tv, h_dst,
                          outT_r[:, :, col0:col0 + w])

        def chunk_big(d, col0, st0, pp0, w, store, buf, ftq, ftb):
            """Full-size chunk (w > TPACK_W): one psum pair per (gate, t)."""
            pbuf = (d - 1) % 2
            hw = w // 2
            for t in range(KT):
                ps_px = pspool.tile([P, CHUNK], F32, tag="ps")
                for k in range(KT):
                    for c0 in range(0, w, 512):
                        cw = min(512, w - c0)
                        nc.tensor.matmul(
                            ps_px[:, c0:c0 + cw],
                            wpx_sb[:, k, t * P:(t + 1) * P],
                            ftb[:, k, c0:c0 + cw],
                            start=(k == 0), stop=(k == KT - 1))
                px = xpool.tile([P, CHUNK], BF16, tag="px")
                nc.scalar.activation(px[:, :w], ps_px[:, :w], AF.Identity,
                                     bias=pxb_sb[:, t:t + 1])

                def gate(gi):
                    m = gi * KT + t
                    ps = pspool.tile([P, CHUNK], F32, tag="ps")
                    for kp in (0, 2):
                        for c0 in range(0, w, 256):
                            cw = min(256, w - c0)
                            nc.tensor.matmul(
                                ps[:, c0:c0 + cw],
                                wx_sb[:, kp:kp + 2, m * P:(m + 1) * P],
                                ftq[:, kp:kp + 2, c0:c0 + cw],
                                perf_mode=DR,
                                start=(kp == 0 and c0 % 512 == 0),
                                stop=False)
                    for kp in (0, 2):
                        for p0 in range(0, hw, P):
                            pw = min(P, hw - p0)
                            hrep = hst[pbuf][
                                :, kp:kp + 2, pp0 + p0:pp0 + p0 + pw,
                                None].to_broadcast((P, 2, pw, 2))
                            c0 = 2 * p0
                            last_h = (kp == 2
                                      and (c0 + 256 >= w or c0 + 256 == 512))
                            nc.tensor.matmul(
                                ps[:, c0:c0 + 2 * pw],
                                wh_sb[:, kp:kp + 2, m * P:(m + 1) * P],
                                hrep, perf_mode=DR,
                                start=False, stop=last_h)
                    g = gpool.tile([P, CHUNK], BF16, tag="gates")
                    func = AF.Tanh if gi == 3 else AF.Sigmoid
                    nc.scalar.activation(g[:, :w], ps[:, :w], func,
                                         bias=bias_sb[:, m:m + 1],
                                         scale=1.0 / WSCALE)
                    return g

                if store:
                    c_dst = cst[buf][:, t, st0:st0 + w]
                    h_dst = hst[buf][:, t, st0:st0 + w]
                else:
                    c_dst = tpool.tile([P, CHUNK], BF16, tag="tmp",
                                       name="ctmp")[:, :w]
                    h_dst = None
                tv = lambda ap: ap
                pc_reps = [(slice(0, w),
                            cst[pbuf][:, t, pp0:pp0 + hw, None].to_broadcast(
                                (P, hw, 2)))]
                gi_ = gate(0)
                gu_ = gate(3)
                gf_ = gate(2)
                tc_ = elemwise_early(d, w, gi_, gu_, gf_, px, pc_reps,
                                     c_dst, tv)
                go_ = gate(1)
                d_ = elemwise_mid(w, go_, tc_, px)
                gr_ = gate(4)
                elemwise_late(w, gr_, d_, px, tv, h_dst,
                              outT_r[:, t, col0:col0 + w])

        def chunk(d, col0, st0, pp0, w, store, buf):
            """One chunk of w child columns at level d (node order).
            col0: feats/outT column base; st0: state store offset; pp0:
            parent offset in the parent level's state buffers."""
            ftq = fpool.tile([P, KT, CHUNK], FP8, tag="ftq")
            ftb = fpool.tile([P, KT, CHUNK], BF16, tag="ftb")
            nc.sync.dma_start(ftq[:, :, :w], featsq_r[:, :, col0:col0 + w])
            nc.sync.dma_start(ftb[:, :, :w], featsb_r[:, :, col0:col0 + w])
            if w <= TPACK_W:
                chunk_tpack(d, col0, st0, pp0, w, store, buf, ftq, ftb)
            else:
                chunk_big(d, col0, st0, pp0, w, store, buf, ftq, ftb)

        for (d, s, l) in segs:
            store = d < depth - 1
            buf = d % 2
            pbase = s // 2 if (split and d == depth - 1) else 0
            for q0 in range(s, s + l, CHUNK):
                w = min(CHUNK, s + l - q0)
                chunk(d, off[d] + q0, q0 - s, q0 // 2 - pbase, w, store, buf)

    nc.compile()
    return nc, C


# ---------------------------------------------------------------- host side

def _col_maps(depth):
    """Per (core, level): global node index for each column (node order)."""
    Ns = _level_sizes(depth)
    maps = []
    for i in range(NCORES):
        per_level = []
        for d in range(depth):
            base = (1 << d) - 1
            if d <= 3:
                orig = np.arange(Ns[d]) ^ (i >> (3 - d))
            else:
                orig = i * (1 << (d - 3)) + np.arange(Ns[d])
            per_level.append(base + orig)
        maps.append(per_level)
    return maps


def prep_inputs(features, px_w, px_b, iofux_w, iofux_b, iofuh_w, iofuh_b,
                depth=DEPTH):
    Ns = _level_sizes(depth)
    C = sum(Ns)
    features = np.asarray(features, np.float32)
    wxq = np.ascontiguousarray(
        (np.asarray(iofux_w, np.float32) * WSCALE).T).astype(np_fp8)
    wxpx = np.ascontiguousarray(
        np.asarray(px_w, np.float32).T).astype(np_bf16)
    whq = np.ascontiguousarray(
        (np.asarray(iofuh_w, np.float32) * WHSCALE).T).astype(np_fp8)
    bias_all = np.asarray(iofux_b, np.float32) + np.asarray(iofuh_b, np.float32)
    biasm = np.ascontiguousarray(bias_all.reshape(M_IOFU, P).T)
    pxbm = np.ascontiguousarray(
        np.asarray(px_b, np.float32).reshape(M_PX, P).T)

    maps = _col_maps(depth)
    in_maps = []
    for i in range(NCORES):
        cols = np.concatenate(maps[i])
        fcore = features[cols, :]                       # [C, 512] f32
        fT = np.ascontiguousarray(fcore.T)              # [512, C]
        in_maps.append({"featsq": fT.astype(np_fp8),
                        "featsb": fT.astype(np_bf16),
                        "wxq": wxq, "wxpx": wxpx, "whq": whq,
                        "biasm": biasm, "pxb": pxbm})
    return in_maps, maps, C


def assemble_output(results, maps, depth=DEPTH):
    Ns = _level_sizes(depth)
    n_nodes = (1 << depth) - 1
    out = np.empty((n_nodes, H), np.float32)
    offs = np.cumsum([0] + Ns)
    for i in range(NCORES):
        o = results[i]["outT"]                          # [512, C] bf16
        for d in range(depth):
            if d <= 3 and i != 0:
                continue  # replicated levels: take core 0's copy
            cols = maps[i][d]
            out[cols, :] = o[:, offs[d]:offs[d + 1]].T.astype(np.float32)
    return out


_CACHE = {}


def _get_built(depth=DEPTH):
    if depth not in _CACHE:
        _CACHE[depth] = build_nc(depth)
    return _CACHE[depth]


def run_cores(in_maps, depth=DEPTH, trace=False):
    from concourse.bass_utils import run_bass_kernel_spmd
    nc, C = _get_built(depth)
    br = run_bass_kernel_spmd(nc, in_maps, list(range(NCORES)), trace=trace)
    return br


def kernel(features, px_w, px_b, iofux_w, iofux_b, iofuh_w, iofuh_b):
    in_maps, maps, C = prep_inputs(features, px_w, px_b, iofux_w, iofux_b,
                                   iofuh_w, iofuh_b)
    br = run_cores(in_maps)
    return assemble_output(br.results, maps)


# revision 29
# speedup vs baseline: 1.9294x; 1.0406x over previous
"""Root-to-leaves TreeLSTM over a complete binary tree (depth 17, 131071 nodes,
feat=h=512), distributed over 8 TRN2 NeuronCores with zero inter-core
communication.

Sharding: level d's nodes split into 8 contiguous chunks means each core's
chunk at level d+1 is exactly the children of its chunk at level d, so each
core owns one of the 8 subtrees rooted at level 3. Levels 0-3 are replicated
on all cores; the SPMD program relabels them per-core by XOR with the core
index prefix so "my subtree root" is column 0 everywhere and the parent map
is position-independent (parent of col j is col j//2 in plain node order).

v2: fp8 DoubleRow matmuls for the iofu x-GEMM and h-GEMM (2x PE throughput),
both accumulated into the same 2-bank PSUM pair per gate; the parent-h moving
operand uses a stride-0 repeat AP so each parent feeds both of its children
in one GEMM (no copies / identity injections / L-R block permutation). The
px GEMM stays bf16 for accuracy (its error passes to the output unattenuated
by any sigmoid). Weights are pre-scaled into fp8's normal range (wx*64,
wh*16, h-state*4) and compensated by the activation scale (1/64). Gates, c
state, tmps and the output are bf16 (2x DVE, half DMA); h state is fp8
(DoubleRow input). Elementwise is split DVE/Pool to balance engines.
"""

import os
import sys

sys.path.insert(0, "/opt/trn_rl_repo")

import numpy as np
import ml_dtypes
from contextlib import ExitStack

import concourse.bass as bass
import concourse.mybir as mybir
import concourse.tile as tile
from concourse import bacc

P = 128
KT = 4               # 512 / 128 contraction tiles
H = 512
F = 512
DEPTH = 17
NCORES = 8
CHUNK = 1024         # children columns per chunk
PCH = 512            # parents per block-order chunk (=> CHUNK children)
M_IOFU = 20          # iofu M-tiles (2560/128), fp8
M_PX = 4             # px M-tiles (512/128), bf16
WSCALE = 64.0        # wx fp8 pre-scale
WHSCALE = 64.0       # wh fp8 pre-scale (h state stored unscaled)
TPACK_W = 256        # pack all 4 k-tiles into one psum pair when w <= this
BF16 = mybir.dt.bfloat16
FP8 = mybir.dt.float8e4
F32 = mybir.dt.float32
AF = mybir.ActivationFunctionType
DR = mybir.MatmulPerfMode.DoubleRow
np_bf16 = ml_dtypes.bfloat16
np_fp8 = ml_dtypes.float8_e4m3


def _level_sizes(depth):
    # per-core column count per level: levels 0..3 replicated, >=4 core-private
    return [1 << d if d <= 3 else 1 << (d - 3) for d in range(depth)]


def _plan(depth):
    """Level sizes, feature-column offsets, and per-parity state-buffer
    widths (level d stores into buffer d % 2; no level splitting — the two
    buffers are sized for the largest even/odd stored level)."""
    Ns = _level_sizes(depth)
    off = [0]
    for n in Ns:
        off.append(off[-1] + n)
    sc = [1, 1]
    for d in range(depth - 1):
        sc[d % 2] = max(sc[d % 2], Ns[d])
    return Ns, off, sc


def build_nc(depth=DEPTH):
    """Build the SPMD single-core Bass program (same NEFF for all 8 cores)."""
    Ns, off, sc = _plan(depth)
    C = off[-1]

    nc = bacc.Bacc("TRN2", target_bir_lowering=False, debug=False)
    featsq = nc.declare_dram_parameter("featsq", [F, C], FP8, isOutput=False)
    featsb = nc.declare_dram_parameter("featsb", [F, C], BF16, isOutput=False)
    wxq = nc.declare_dram_parameter("wxq", [F, M_IOFU * P], FP8, isOutput=False)
    wxpx = nc.declare_dram_parameter("wxpx", [F, M_PX * P], BF16, isOutput=False)
    whq = nc.declare_dram_parameter("whq", [H, M_IOFU * P], FP8, isOutput=False)
    biasm = nc.declare_dram_parameter("biasm", [P, M_IOFU], F32, isOutput=False)
    pxb = nc.declare_dram_parameter("pxb", [P, M_PX], F32, isOutput=False)
    ident = nc.declare_dram_parameter("ident", [P, P], BF16, isOutput=False)
    outT = nc.declare_dram_parameter("outT", [H, C], BF16, isOutput=True)

    featsq_r = featsq[:].rearrange("(a p) c -> p a c", p=P)
    featsb_r = featsb[:].rearrange("(a p) c -> p a c", p=P)
    wxq_r = wxq[:].rearrange("(a p) m -> p a m", p=P)
    wxpx_r = wxpx[:].rearrange("(a p) m -> p a m", p=P)
    whq_r = whq[:].rearrange("(a p) m -> p a m", p=P)
    outT_r = outT[:].rearrange("(a p) c -> p a c", p=P)

    with ExitStack() as ctx:
        tc = ctx.enter_context(tile.TileContext(nc))
        wpool = ctx.enter_context(tc.tile_pool(name="w", bufs=1))
        spool = ctx.enter_context(tc.tile_pool(name="state", bufs=1))
        fpool = ctx.enter_context(tc.tile_pool(name="feats", bufs=3))
        pspool = ctx.enter_context(tc.tile_pool(name="ps", bufs=4, space="PSUM"))
        gpool = ctx.enter_context(tc.tile_pool(name="gates", bufs=10))
        xpool = ctx.enter_context(tc.tile_pool(name="px", bufs=4))
        tpool = ctx.enter_context(tc.tile_pool(name="tmp", bufs=8))
        opool = ctx.enter_context(tc.tile_pool(name="hf", bufs=4))
        ghpool = ctx.enter_context(tc.tile_pool(name="gh", bufs=3))

        wx_sb = wpool.tile([P, KT, M_IOFU * P], FP8, tag="wxq")
        wpx_sb = wpool.tile([P, KT, M_PX * P], BF16, tag="wxpx")
        wh_sb = wpool.tile([P, KT, M_IOFU * P], FP8, tag="whq")
        bias_sb = wpool.tile([P, M_IOFU], F32, tag="biasm")
        pxb_sb = wpool.tile([P, M_PX], F32, tag="pxb")
        id_sb = wpool.tile([P, P], BF16, tag="ident")
        nc.sync.dma_start(wx_sb[:], wxq_r)
        nc.sync.dma_start(wpx_sb[:], wxpx_r)
        nc.sync.dma_start(wh_sb[:], whq_r)
        nc.sync.dma_start(bias_sb[:], biasm[:])
        nc.sync.dma_start(pxb_sb[:], pxb[:])
        nc.sync.dma_start(id_sb[:], ident[:])

        # state double buffers (level d -> buffer d % 2): c bf16, h fp8
        cst = [spool.tile([P, KT, sc[b]], BF16, tag=f"c{b}", name=f"c{b}")
               for b in (0, 1)]
        hst = [spool.tile([P, KT, sc[b]], FP8, tag=f"h{b}", name=f"h{b}")
               for b in (0, 1)]

        def elemwise_early(d, w, gi_, gu_, gf_, px, pc_reps, c_dst, tv):
            """First part of the bf16 elementwise chain (after i/u/f acts):
            c = i*u + f*pc and tanh(c). Runs while o/r GEMMs proceed."""
            if d > 0:
                t1 = tpool.tile([P, CHUNK], BF16, tag="tmp")
                t2 = tpool.tile([P, CHUNK], BF16, tag="tmp")
                nc.vector.tensor_mul(t1[:, :w], gi_[:, :w], gu_[:, :w])
                for sl, vfn, pc_rep in pc_reps:
                    nc.gpsimd.tensor_mul(vfn(t2[:, sl]), vfn(gf_[:, sl]),
                                         pc_rep)
                nc.vector.tensor_add(c_dst, tv(t1[:, :w]), tv(t2[:, :w]))
            else:
                nc.vector.tensor_mul(c_dst, tv(gi_[:, :w]), tv(gu_[:, :w]))
            tc_ = tpool.tile([P, CHUNK], BF16, tag="tmp")
            nc.scalar.activation(tv(tc_[:, :w]), c_dst, AF.Tanh)
            return tc_

        def elemwise_mid(w, go_, tc_, px):
            """After o's act: t3 = o*tanh(c); d = t3 - px."""
            t3 = tpool.tile([P, CHUNK], BF16, tag="tmp")
            nc.vector.tensor_mul(t3[:, :w], go_[:, :w], tc_[:, :w])
            d_ = tpool.tile([P, CHUNK], BF16, tag="tmp")
            nc.vector.tensor_sub(d_[:, :w], t3[:, :w], px[:, :w])
            return d_

        def elemwise_late(w, gr_, d_, px, tv, h_dst, out_ap):
            """After r's act (critical tail): e = r*d; hf = e + px; store."""
            e_ = tpool.tile([P, CHUNK], BF16, tag="tmp")
            nc.vector.tensor_mul(e_[:, :w], gr_[:, :w], d_[:, :w])
            hf = opool.tile([P, CHUNK], BF16, tag="hf")
            nc.vector.tensor_add(hf[:, :w], e_[:, :w], px[:, :w])
            if h_dst is not None:
                nc.vector.tensor_copy(h_dst, tv(hf[:, :w]))
            nc.sync.dma_start(out_ap, tv(hf[:, :w]))

        def chunk_tpack(d, col0, st0, pp0, w, store, buf, ftq, ftb):
            """Small-level chunk (w <= TPACK_W): all 4 k-tiles packed into
            one psum pair per gate -> one activation per gate, 3-dim state
            APs, 4x fewer elementwise/activation ops."""
            pbuf = (d - 1) % 2
            hw = w // 2
            tw = KT * w
            tpb = max(1, 512 // w)  # t-slices per 2KB psum bank

            def first_t(t):
                return t % tpb == 0

            def last_t(t):
                return t % tpb == tpb - 1 or t == KT - 1

            ps_px = pspool.tile([P, CHUNK], F32, tag="ps")
            for t in range(KT):
                for k in range(KT):
                    nc.tensor.matmul(
                        ps_px[:, t * w:(t + 1) * w],
                        wpx_sb[:, k, t * P:(t + 1) * P], ftb[:, k, :w],
                        start=(first_t(t) and k == 0),
                        stop=(last_t(t) and k == KT - 1))
            px = xpool.tile([P, CHUNK], BF16, tag="px")
            nc.scalar.activation(px[:, :tw], ps_px[:, :tw], AF.Identity,
                                 bias=pxb_sb[:, 0:1])

            def gate(gi):
                ps = pspool.tile([P, CHUNK], F32, tag="ps")
                for t in range(KT):
                    m = gi * KT + t
                    for kp in (0, 2):
                        nc.tensor.matmul(
                            ps[:, t * w:(t + 1) * w],
                            wx_sb[:, kp:kp + 2, m * P:(m + 1) * P],
                            ftq[:, kp:kp + 2, :w], perf_mode=DR,
                            start=(first_t(t) and kp == 0),
                            stop=(last_t(t) and kp == 2 and d == 0))
                    if d > 0:
                        for kp in (0, 2):
                            hrep = hst[pbuf][
                                :, kp:kp + 2, pp0:pp0 + hw,
                                None].to_broadcast((P, 2, hw, 2))
                            nc.tensor.matmul(
                                ps[:, t * w:t * w + 2 * hw],
                                wh_sb[:, kp:kp + 2, m * P:(m + 1) * P],
                                hrep, perf_mode=DR,
                                start=False, stop=(last_t(t) and kp == 2))
                g = gpool.tile([P, CHUNK], BF16, tag="gates")
                func = AF.Tanh if gi == 3 else AF.Sigmoid
                nc.scalar.activation(g[:, :tw], ps[:, :tw], func,
                                     bias=bias_sb[:, gi * KT:gi * KT + 1],
                                     scale=1.0 / WSCALE)
                return g

            if store:
                c_dst = cst[buf][:, :, st0:st0 + w]
                h_dst = hst[buf][:, :, st0:st0 + w]
            else:
                c_dst = tpool.tile([P, CHUNK], BF16, tag="tmp",
                                   name="ctmp")[:, :tw].rearrange(
                                       "p (a b) -> p a b", a=KT)
                h_dst = None
            tv = lambda ap: ap.rearrange("p (a b) -> p a b", a=KT)
            nodev = lambda ap: ap.rearrange("p (a b) -> p a b", b=2)
            pc_reps = []
            if d > 0:
                for t in range(KT):
                    pc_reps.append((
                        slice(t * w, (t + 1) * w), nodev,
                        cst[pbuf][:, t, pp0:pp0 + hw, None].to_broadcast(
                            (P, hw, 2))))
            gi_ = gate(0)
            gu_ = gate(3)
            gf_ = gate(2)
            tc_ = elemwise_early(d, tw, gi_, gu_, gf_, px, pc_reps, c_dst, tv)
            go_ = gate(1)
            d_ = elemwise_mid(tw, go_, tc_, px)
            gr_ = gate(4)
            elemwise_late(tw, gr_, d_, px, tv, h_dst,
                          outT_r[:, :, col0:col0 + w])

        def chunk_blk(d, coloff, npar, p0, pw, store, buf):
            """Block-order chunk for big levels (level size > TPACK_W):
            process parents [p0, p0+pw); children laid out [L-block |
            R-block] within the level. Per gate: h-GEMM once into the L
            half of a 2-bank psum pair, R x-GEMM into the R half as its
            own group, one DVE add seeds gh into R, then the L x-GEMM
            accumulates on top of gh. Halves the h-GEMM tensor work."""
            pbuf = (d - 1) % 2
            w2 = 2 * pw
            ftq = fpool.tile([P, KT, CHUNK], FP8, tag="ftq")
            ftb = fpool.tile([P, KT, CHUNK], BF16, tag="ftb")
            cL = coloff + p0
            cR = coloff + npar + p0
            nc.sync.dma_start(ftq[:, :, 0:pw], featsq_r[:, :, cL:cL + pw])
            nc.sync.dma_start(ftq[:, :, pw:w2], featsq_r[:, :, cR:cR + pw])
            nc.sync.dma_start(ftb[:, :, 0:pw], featsb_r[:, :, cL:cL + pw])
            nc.sync.dma_start(ftb[:, :, pw:w2], featsb_r[:, :, cR:cR + pw])
            for t in range(KT):
                ps_px = pspool.tile([P, CHUNK], F32, tag="ps")
                for k in range(KT):
                    for c0 in range(0, w2, 512):
                        cw = min(512, w2 - c0)
                        nc.tensor.matmul(
                            ps_px[:, c0:c0 + cw],
                            wpx_sb[:, k, t * P:(t + 1) * P],
                            ftb[:, k, c0:c0 + cw],
                            start=(k == 0 and c0 % 512 == 0),
                            stop=(k == KT - 1
                                  and (c0 + cw == w2 or (c0 + cw) % 512 == 0)))
                px = xpool.tile([P, CHUNK], BF16, tag="px")
                nc.scalar.activation(px[:, :w2], ps_px[:, :w2], AF.Identity,
                                     bias=pxb_sb[:, t:t + 1])

                def gate_h_rx(gi, ps):
                    """h-GEMM into [0:pw] (group opened, not closed) and the
                    R-children x-GEMM into [pw:w2]."""
                    m = gi * KT + t
                    for kp in (0, 2):
                        for q0 in range(0, pw, 256):
                            qw = min(256, pw - q0)
                            nc.tensor.matmul(
                                ps[:, q0:q0 + qw],
                                wh_sb[:, kp:kp + 2, m * P:(m + 1) * P],
                                hst[pbuf][:, kp:kp + 2, p0 + q0:p0 + q0 + qw],
                                perf_mode=DR,
                                start=(kp == 0 and q0 % 512 == 0),
                                stop=False)
                    for kp in (0, 2):
                        for c0 in range(pw, w2, 256):
                            cw = min(256, w2 - c0)
                            nc.tensor.matmul(
                                ps[:, c0:c0 + cw],
                                wx_sb[:, kp:kp + 2, m * P:(m + 1) * P],
                                ftq[:, kp:kp + 2, c0:c0 + cw],
                                perf_mode=DR,
                                start=(kp == 0 and c0 % 512 == 0),
                                stop=False)

                def gate_lx(gi, ps):
                    """L-children x-GEMM accumulating onto gh in [0:pw];
                    closes the bank(s) the h-GEMM opened."""
                    m = gi * KT + t
                    for kp in (0, 2):
                        for c0 in range(0, pw, 256):
                            cw = min(256, pw - c0)
                            nc.tensor.matmul(
                                ps[:, c0:c0 + cw],
                                wx_sb[:, kp:kp + 2, m * P:(m + 1) * P],
                                ftq[:, kp:kp + 2, c0:c0 + cw],
                                perf_mode=DR, start=False,
                                stop=(kp == 2 and (c0 + cw == pw
                                                   or (c0 + cw) % 512 == 0)))

                def gate_act(gi, ps):
                    g = gpool.tile([P, CHUNK], BF16, tag="gates")
                    func = AF.Tanh if gi == 3 else AF.Sigmoid
                    nc.scalar.activation(g[:, :w2], ps[:, :w2], func,
                                         bias=bias_sb[:, gi * KT:gi * KT + 1],
                                         scale=1.0 / WSCALE)
                    return g

                if store:
                    c_dst = cst[buf][:, t, 0:2 * npar].rearrange(
                        "p (b q) -> p b q", b=2)[:, :, p0:p0 + pw]
                    h_dst = hst[buf][:, t, 0:2 * npar].rearrange(
                        "p (b q) -> p b q", b=2)[:, :, p0:p0 + pw]
                else:
                    c_dst = tpool.tile([P, CHUNK], BF16, tag="tmp",
                                       name="ctmp")[:, :w2].rearrange(
                                           "p (b q) -> p b q", b=2)
                    h_dst = None
                tv = lambda ap: ap.rearrange("p (b q) -> p b q", b=2)
                out_ap = outT_r[:, t, coloff:coloff + 2 * npar].rearrange(
                    "p (b q) -> p b q", b=2)[:, :, p0:p0 + pw]
                blkv = lambda ap: ap.rearrange("p (b q) -> p b q", b=2)
                pc_reps = [(slice(0, w2), blkv,
                            cst[pbuf][:, t, None, p0:p0 + pw].to_broadcast(
                                (P, 2, pw)))]

                def inject(gi):
                    # accumulate the 64-scaled gh into the R half via an
                    # identity matmul from the SBUF copy
                    nc.tensor.matmul(pss[gi][:, pw:w2], id_sb[:], ghs[gi],
                                     start=False, stop=(pw % 512 == 0))

                # software-pipelined gate sequence: i, u, f, o, r
                acted = {}
                order = [0, 3, 2, 1, 4]
                pss = {}
                ghs = {}
                prev = None
                for gi in order:
                    ps = pspool.tile([P, CHUNK], F32, tag="ps")
                    pss[gi] = ps
                    gate_h_rx(gi, ps)
                    gh = ghpool.tile([P, PCH], BF16, tag="gh",
                                     name="gh")[:, :pw]
                    nc.vector.tensor_copy(gh, ps[:, 0:pw])
                    ghs[gi] = gh
                    if prev is not None:
                        inject(prev)
                        gate_lx(prev, pss[prev])
                        acted[prev] = gate_act(prev, pss[prev])
                    prev = gi
                inject(prev)
                gate_lx(prev, pss[prev])
                acted[prev] = gate_act(prev, pss[prev])
                tc_ = elemwise_early(d, w2, acted[0], acted[3], acted[2],
                                     px, pc_reps, c_dst, tv)
                d_ = elemwise_mid(w2, acted[1], tc_, px)
                elemwise_late(w2, acted[4], d_, px, tv, h_dst, out_ap)

        for d in range(depth):
            store = d < depth - 1
            buf = d % 2
            n = Ns[d]
            if n <= TPACK_W:
                ftq = fpool.tile([P, KT, CHUNK], FP8, tag="ftq")
                ftb = fpool.tile([P, KT, CHUNK], BF16, tag="ftb")
                nc.sync.dma_start(ftq[:, :, :n],
                                  featsq_r[:, :, off[d]:off[d] + n])
                nc.sync.dma_start(ftb[:, :, :n],
                                  featsb_r[:, :, off[d]:off[d] + n])
                chunk_tpack(d, off[d], 0, 0, n, store, buf, ftq, ftb)
            else:
                npar = n // 2
                for p0 in range(0, npar, PCH):
                    pw = min(PCH, npar - p0)
                    chunk_blk(d, off[d], npar, p0, pw, store, buf)

    nc.compile()
    return nc, C


# ---------------------------------------------------------------- host side

def _col_maps(depth):
    """Per (core, level): global node index for each column. Small levels
    (size <= TPACK_W) use node order (children interleaved: parent of col j
    is col j//2); big levels use block order ([L-children | R-children]:
    parent of col j is col j mod npar). Levels 0-3 are replicated with a
    per-core child-swap so each core's subtree root lands at column 0 of
    level 3."""
    Ns = _level_sizes(depth)
    maps = []
    for i in range(NCORES):
        per_level = []
        cur = np.array([0], dtype=np.int64)
        for d in range(depth):
            if d == 0:
                cur = np.array([0], dtype=np.int64)
            elif d == 4:
                # first private level: children of this core's subtree root
                r = cur[0]
                cur = np.array([2 * r, 2 * r + 1], dtype=np.int64)
            else:
                L, R = 2 * cur, 2 * cur + 1
                if d <= 3:
                    if (i >> (3 - d)) & 1:
                        L, R = R, L
                    cur = np.stack([L, R], axis=1).ravel()
                elif Ns[d] <= TPACK_W:
                    cur = np.stack([L, R], axis=1).ravel()
                else:
                    cur = np.concatenate([L, R])
            per_level.append(((1 << d) - 1) + cur)
        maps.append(per_level)
    return maps


def prep_inputs(features, px_w, px_b, iofux_w, iofux_b, iofuh_w, iofuh_b,
                depth=DEPTH):
    Ns = _level_sizes(depth)
    C = sum(Ns)
    features = np.asarray(features, np.float32)
    wxq = np.ascontiguousarray(
        (np.asarray(iofux_w, np.float32) * WSCALE).T).astype(np_fp8)
    wxpx = np.ascontiguousarray(
        np.asarray(px_w, np.float32).T).astype(np_bf16)
    whq = np.ascontiguousarray(
        (np.asarray(iofuh_w, np.float32) * WHSCALE).T).astype(np_fp8)
    bias_all = np.asarray(iofux_b, np.float32) + np.asarray(iofuh_b, np.float32)
    biasm = np.ascontiguousarray(bias_all.reshape(M_IOFU, P).T)
    pxbm = np.ascontiguousarray(
        np.asarray(px_b, np.float32).reshape(M_PX, P).T)

    maps = _col_maps(depth)
    idm = np.eye(P, dtype=np_bf16)
    in_maps = []
    for i in range(NCORES):
        cols = np.concatenate(maps[i])
        fcore = features[cols, :]                       # [C, 512] f32
        fT = np.ascontiguousarray(fcore.T)              # [512, C]
        in_maps.append({"featsq": fT.astype(np_fp8),
                        "featsb": fT.astype(np_bf16),
                        "wxq": wxq, "wxpx": wxpx, "whq": whq,
                        "biasm": biasm, "pxb": pxbm, "ident": idm})
    return in_maps, maps, C


def assemble_output(results, maps, depth=DEPTH):
    Ns = _level_sizes(depth)
    n_nodes = (1 << depth) - 1
    out = np.empty((n_nodes, H), np.float32)
    offs = np.cumsum([0] + Ns)
    for i in range(NCORES):
        o = results[i]["outT"]                          # [512, C] bf16
        for d in range(depth):
            if d <= 3 and i != 0:
                continue  # replicated levels: take core 0's copy
            cols = maps[i][d]
            out[cols, :] = o[:, offs[d]:offs[d + 1]].T.astype(np.float32)
    return out


_CACHE = {}


def _get_built(depth=DEPTH):
    if depth not in _CACHE:
        _CACHE[depth] = build_nc(depth)
    return _CACHE[depth]


def run_cores(in_maps, depth=DEPTH, trace=False):
    from concourse.bass_utils import run_bass_kernel_spmd
    nc, C = _get_built(depth)
    br = run_bass_kernel_spmd(nc, in_maps, list(range(NCORES)), trace=trace)
    return br


def kernel(features, px_w, px_b, iofux_w, iofux_b, iofuh_w, iofuh_b):
    in_maps, maps, C = prep_inputs(features, px_w, px_b, iofux_w, iofux_b,
                                   iofuh_w, iofuh_b)
    br = run_cores(in_maps)
    return assemble_output(br.results, maps)


# revision 31
# speedup vs baseline: 1.9296x; 1.0001x over previous
"""Root-to-leaves TreeLSTM over a complete binary tree (depth 17, 131071 nodes,
feat=h=512), distributed over 8 TRN2 NeuronCores with zero inter-core
communication.

Sharding: level d's nodes split into 8 contiguous chunks means each core's
chunk at level d+1 is exactly the children of its chunk at level d, so each
core owns one of the 8 subtrees rooted at level 3. Levels 0-3 are replicated
on all cores; the SPMD program relabels them per-core by XOR with the core
index prefix so "my subtree root" is column 0 everywhere and the parent map
is position-independent (parent of col j is col j//2 in plain node order).

v2: fp8 DoubleRow matmuls for the iofu x-GEMM and h-GEMM (2x PE throughput),
both accumulated into the same 2-bank PSUM pair per gate; the parent-h moving
operand uses a stride-0 repeat AP so each parent feeds both of its children
in one GEMM (no copies / identity injections / L-R block permutation). The
px GEMM stays bf16 for accuracy (its error passes to the output unattenuated
by any sigmoid). Weights are pre-scaled into fp8's normal range (wx*64,
wh*16, h-state*4) and compensated by the activation scale (1/64). Gates, c
state, tmps and the output are bf16 (2x DVE, half DMA); h state is fp8
(DoubleRow input). Elementwise is split DVE/Pool to balance engines.
"""

import os
import sys

sys.path.insert(0, "/opt/trn_rl_repo")

import numpy as np
import ml_dtypes
from contextlib import ExitStack

import concourse.bass as bass
import concourse.mybir as mybir
import concourse.tile as tile
from concourse import bacc

P = 128
KT = 4               # 512 / 128 contraction tiles
H = 512
F = 512
DEPTH = 17
NCORES = 8
CHUNK = 1024         # children columns per chunk
PCH = 512            # parents per block-order chunk (=> CHUNK children)
M_IOFU = 20          # iofu M-tiles (2560/128), fp8
M_PX = 4             # px M-tiles (512/128), bf16
WSCALE = 64.0        # wx fp8 pre-scale
WHSCALE = 64.0       # wh fp8 pre-scale (h state stored unscaled)
TPACK_W = 256        # pack all 4 k-tiles into one psum pair when w <= this
BF16 = mybir.dt.bfloat16
FP8 = mybir.dt.float8e4
F32 = mybir.dt.float32
AF = mybir.ActivationFunctionType
DR = mybir.MatmulPerfMode.DoubleRow
np_bf16 = ml_dtypes.bfloat16
np_fp8 = ml_dtypes.float8_e4m3


def _level_sizes(depth):
    # per-core column count per level: levels 0..3 replicated, >=4 core-private
    return [1 << d if d <= 3 else 1 << (d - 3) for d in range(depth)]


def _plan(depth):
    """Level sizes, feature-column offsets, and per-parity state-buffer
    widths (level d stores into buffer d % 2; no level splitting — the two
    buffers are sized for the largest even/odd stored level)."""
    Ns = _level_sizes(depth)
    off = [0]
    for n in Ns:
        off.append(off[-1] + n)
    sc = [1, 1]
    for d in range(depth - 1):
        sc[d % 2] = max(sc[d % 2], Ns[d])
    return Ns, off, sc


def build_nc(depth=DEPTH):
    """Build the SPMD single-core Bass program (same NEFF for all 8 cores)."""
    Ns, off, sc = _plan(depth)
    C = off[-1]

    nc = bacc.Bacc("TRN2", target_bir_lowering=False, debug=False)
    featsq = nc.declare_dram_parameter("featsq", [F, C], FP8, isOutput=False)
    featsb = nc.declare_dram_parameter("featsb", [F, C], BF16, isOutput=False)
    wxq = nc.declare_dram_parameter("wxq", [F, M_IOFU * P], FP8, isOutput=False)
    wxpx = nc.declare_dram_parameter("wxpx", [F, M_PX * P], BF16, isOutput=False)
    whq = nc.declare_dram_parameter("whq", [H, M_IOFU * P], FP8, isOutput=False)
    biasm = nc.declare_dram_parameter("biasm", [P, M_IOFU], F32, isOutput=False)
    pxb = nc.declare_dram_parameter("pxb", [P, M_PX], F32, isOutput=False)
    ident = nc.declare_dram_parameter("ident", [P, P], BF16, isOutput=False)
    outT = nc.declare_dram_parameter("outT", [H, C], BF16, isOutput=True)

    featsq_r = featsq[:].rearrange("(a p) c -> p a c", p=P)
    featsb_r = featsb[:].rearrange("(a p) c -> p a c", p=P)
    wxq_r = wxq[:].rearrange("(a p) m -> p a m", p=P)
    wxpx_r = wxpx[:].rearrange("(a p) m -> p a m", p=P)
    whq_r = whq[:].rearrange("(a p) m -> p a m", p=P)
    outT_r = outT[:].rearrange("(a p) c -> p a c", p=P)

    with ExitStack() as ctx:
        tc = ctx.enter_context(tile.TileContext(nc))
        wpool = ctx.enter_context(tc.tile_pool(name="w", bufs=1))
        spool = ctx.enter_context(tc.tile_pool(name="state", bufs=1))
        fpool = ctx.enter_context(tc.tile_pool(name="feats", bufs=3))
        pspool = ctx.enter_context(tc.tile_pool(name="ps", bufs=4, space="PSUM"))
        gpool = ctx.enter_context(tc.tile_pool(name="gates", bufs=10))
        xpool = ctx.enter_context(tc.tile_pool(name="px", bufs=4))
        tpool = ctx.enter_context(tc.tile_pool(name="tmp", bufs=8))
        opool = ctx.enter_context(tc.tile_pool(name="hf", bufs=4))
        ghpool = ctx.enter_context(tc.tile_pool(name="gh", bufs=4))

        wx_sb = wpool.tile([P, KT, M_IOFU * P], FP8, tag="wxq")
        wpx_sb = wpool.tile([P, KT, M_PX * P], BF16, tag="wxpx")
        wh_sb = wpool.tile([P, KT, M_IOFU * P], FP8, tag="whq")
        bias_sb = wpool.tile([P, M_IOFU], F32, tag="biasm")
        pxb_sb = wpool.tile([P, M_PX], F32, tag="pxb")
        id_sb = wpool.tile([P, P], BF16, tag="ident")
        nc.sync.dma_start(wx_sb[:], wxq_r)
        nc.sync.dma_start(wpx_sb[:], wxpx_r)
        nc.sync.dma_start(wh_sb[:], whq_r)
        nc.sync.dma_start(bias_sb[:], biasm[:])
        nc.sync.dma_start(pxb_sb[:], pxb[:])
        nc.sync.dma_start(id_sb[:], ident[:])

        # state double buffers (level d -> buffer d % 2): c bf16, h fp8
        cst = [spool.tile([P, KT, sc[b]], BF16, tag=f"c{b}", name=f"c{b}")
               for b in (0, 1)]
        hst = [spool.tile([P, KT, sc[b]], FP8, tag=f"h{b}", name=f"h{b}")
               for b in (0, 1)]

        def elemwise_early(d, w, gi_, gu_, gf_, px, pc_reps, c_dst, tv):
            """First part of the bf16 elementwise chain (after i/u/f acts):
            c = i*u + f*pc and tanh(c). Runs while o/r GEMMs proceed."""
            if d > 0:
                t1 = tpool.tile([P, CHUNK], BF16, tag="tmp")
                t2 = tpool.tile([P, CHUNK], BF16, tag="tmp")
                nc.vector.tensor_mul(t1[:, :w], gi_[:, :w], gu_[:, :w])
                for sl, vfn, pc_rep in pc_reps:
                    nc.gpsimd.tensor_mul(vfn(t2[:, sl]), vfn(gf_[:, sl]),
                                         pc_rep)
                nc.vector.tensor_add(c_dst, tv(t1[:, :w]), tv(t2[:, :w]))
            else:
                nc.vector.tensor_mul(c_dst, tv(gi_[:, :w]), tv(gu_[:, :w]))
            tc_ = tpool.tile([P, CHUNK], BF16, tag="tmp")
            nc.scalar.activation(tv(tc_[:, :w]), c_dst, AF.Tanh)
            return tc_

        def elemwise_mid(w, go_, tc_, px):
            """After o's act: t3 = o*tanh(c); d = t3 - px."""
            t3 = tpool.tile([P, CHUNK], BF16, tag="tmp")
            nc.vector.tensor_mul(t3[:, :w], go_[:, :w], tc_[:, :w])
            d_ = tpool.tile([P, CHUNK], BF16, tag="tmp")
            nc.vector.tensor_sub(d_[:, :w], t3[:, :w], px[:, :w])
            return d_

        def elemwise_late(w, gr_, d_, px, tv, h_dst, out_ap):
            """After r's act (critical tail): e = r*d; hf = e + px; store."""
            e_ = tpool.tile([P, CHUNK], BF16, tag="tmp")
            nc.vector.tensor_mul(e_[:, :w], gr_[:, :w], d_[:, :w])
            hf = opool.tile([P, CHUNK], BF16, tag="hf")
            nc.vector.tensor_add(hf[:, :w], e_[:, :w], px[:, :w])
            if h_dst is not None:
                nc.vector.tensor_copy(h_dst, tv(hf[:, :w]))
            nc.sync.dma_start(out_ap, tv(hf[:, :w]))

        def chunk_tpack(d, col0, st0, pp0, w, store, buf, ftq, ftb):
            """Small-level chunk (w <= TPACK_W): all 4 k-tiles packed into
            one psum pair per gate -> one activation per gate, 3-dim state
            APs, 4x fewer elementwise/activation ops."""
            pbuf = (d - 1) % 2
            hw = w // 2
            tw = KT * w
            tpb = max(1, 512 // w)  # t-slices per 2KB psum bank

            def first_t(t):
                return t % tpb == 0

            def last_t(t):
                return t % tpb == tpb - 1 or t == KT - 1

            ps_px = pspool.tile([P, CHUNK], F32, tag="ps")
            for t in range(KT):
                for k in range(KT):
                    nc.tensor.matmul(
                        ps_px[:, t * w:(t + 1) * w],
                        wpx_sb[:, k, t * P:(t + 1) * P], ftb[:, k, :w],
                        start=(first_t(t) and k == 0),
                        stop=(last_t(t) and k == KT - 1))
            px = xpool.tile([P, CHUNK], BF16, tag="px")
            nc.scalar.activation(px[:, :tw], ps_px[:, :tw], AF.Identity,
                                 bias=pxb_sb[:, 0:1])

            def gate(gi):
                ps = pspool.tile([P, CHUNK], F32, tag="ps")
                for t in range(KT):
                    m = gi * KT + t
                    for kp in (0, 2):
                        nc.tensor.matmul(
                            ps[:, t * w:(t + 1) * w],
                            wx_sb[:, kp:kp + 2, m * P:(m + 1) * P],
                            ftq[:, kp:kp + 2, :w], perf_mode=DR,
                            start=(first_t(t) and kp == 0),
                            stop=(last_t(t) and kp == 2 and d == 0))
                    if d > 0:
                        for kp in (0, 2):
                            hrep = hst[pbuf][
                                :, kp:kp + 2, pp0:pp0 + hw,
                                None].to_broadcast((P, 2, hw, 2))
                            nc.tensor.matmul(
                                ps[:, t * w:t * w + 2 * hw],
                                wh_sb[:, kp:kp + 2, m * P:(m + 1) * P],
                                hrep, perf_mode=DR,
                                start=False, stop=(last_t(t) and kp == 2))
                g = gpool.tile([P, CHUNK], BF16, tag="gates")
                func = AF.Tanh if gi == 3 else AF.Sigmoid
                nc.scalar.activation(g[:, :tw], ps[:, :tw], func,
                                     bias=bias_sb[:, gi * KT:gi * KT + 1],
                                     scale=1.0 / WSCALE)
                return g

            if store:
                c_dst = cst[buf][:, :, st0:st0 + w]
                h_dst = hst[buf][:, :, st0:st0 + w]
            else:
                c_dst = tpool.tile([P, CHUNK], BF16, tag="tmp",
                                   name="ctmp")[:, :tw].rearrange(
                                       "p (a b) -> p a b", a=KT)
                h_dst = None
            tv = lambda ap: ap.rearrange("p (a b) -> p a b", a=KT)
            nodev = lambda ap: ap.rearrange("p (a b) -> p a b", b=2)
            pc_reps = []
            if d > 0:
                for t in range(KT):
                    pc_reps.append((
                        slice(t * w, (t + 1) * w), nodev,
                        cst[pbuf][:, t, pp0:pp0 + hw, None].to_broadcast(
                            (P, hw, 2))))
            gi_ = gate(0)
            gu_ = gate(3)
            gf_ = gate(2)
            tc_ = elemwise_early(d, tw, gi_, gu_, gf_, px, pc_reps, c_dst, tv)
            go_ = gate(1)
            d_ = elemwise_mid(tw, go_, tc_, px)
            gr_ = gate(4)
            elemwise_late(tw, gr_, d_, px, tv, h_dst,
                          outT_r[:, :, col0:col0 + w])

        def chunk_blk(d, coloff, npar, p0, pw, store, buf):
            """Block-order chunk for big levels (level size > TPACK_W):
            process parents [p0, p0+pw); children laid out [L-block |
            R-block] within the level. Per gate: h-GEMM once into the L
            half of a 2-bank psum pair, R x-GEMM into the R half as its
            own group, one DVE add seeds gh into R, then the L x-GEMM
            accumulates on top of gh. Halves the h-GEMM tensor work."""
            pbuf = (d - 1) % 2
            w2 = 2 * pw
            ftq = fpool.tile([P, KT, CHUNK], FP8, tag="ftq")
            ftb = fpool.tile([P, KT, CHUNK], BF16, tag="ftb")
            cL = coloff + p0
            cR = coloff + npar + p0
            nc.sync.dma_start(ftq[:, :, 0:pw], featsq_r[:, :, cL:cL + pw])
            nc.sync.dma_start(ftq[:, :, pw:w2], featsq_r[:, :, cR:cR + pw])
            nc.sync.dma_start(ftb[:, :, 0:pw], featsb_r[:, :, cL:cL + pw])
            nc.sync.dma_start(ftb[:, :, pw:w2], featsb_r[:, :, cR:cR + pw])
            for t in range(KT):
                ps_px = pspool.tile([P, CHUNK], F32, tag="ps")
                for k in range(KT):
                    for c0 in range(0, w2, 512):
                        cw = min(512, w2 - c0)
                        nc.tensor.matmul(
                            ps_px[:, c0:c0 + cw],
                            wpx_sb[:, k, t * P:(t + 1) * P],
                            ftb[:, k, c0:c0 + cw],
                            start=(k == 0 and c0 % 512 == 0),
                            stop=(k == KT - 1
                                  and (c0 + cw == w2 or (c0 + cw) % 512 == 0)))
                px = xpool.tile([P, CHUNK], BF16, tag="px")
                nc.scalar.activation(px[:, :w2], ps_px[:, :w2], AF.Identity,
                                     bias=pxb_sb[:, t:t + 1])

                def gate_h_rx(gi, ps):
                    """h-GEMM into [0:pw] (group opened, not closed) and the
                    R-children x-GEMM into [pw:w2]."""
                    m = gi * KT + t
                    for kp in (0, 2):
                        for q0 in range(0, pw, 256):
                            qw = min(256, pw - q0)
                            nc.tensor.matmul(
                                ps[:, q0:q0 + qw],
                                wh_sb[:, kp:kp + 2, m * P:(m + 1) * P],
                                hst[pbuf][:, kp:kp + 2, p0 + q0:p0 + q0 + qw],
                                perf_mode=DR,
                                start=(kp == 0 and q0 % 512 == 0),
                                stop=False)
                    for kp in (0, 2):
                        for c0 in range(pw, w2, 256):
                            cw = min(256, w2 - c0)
                            nc.tensor.matmul(
                                ps[:, c0:c0 + cw],
                                wx_sb[:, kp:kp + 2, m * P:(m + 1) * P],
                                ftq[:, kp:kp + 2, c0:c0 + cw],
                                perf_mode=DR,
                                start=(kp == 0 and c0 % 512 == 0),
                                stop=False)

                def gate_lx(gi, ps):
                    """L-children x-GEMM accumulating onto gh in [0:pw];
                    closes the bank(s) the h-GEMM opened."""
                    m = gi * KT + t
                    for kp in (0, 2):
                        for c0 in range(0, pw, 256):
                            cw = min(256, pw - c0)
                            nc.tensor.matmul(
                                ps[:, c0:c0 + cw],
                                wx_sb[:, kp:kp + 2, m * P:(m + 1) * P],
                                ftq[:, kp:kp + 2, c0:c0 + cw],
                                perf_mode=DR, start=False,
                                stop=(kp == 2 and (c0 + cw == pw
                                                   or (c0 + cw) % 512 == 0)))

                def gate_act(gi, ps):
                    g = gpool.tile([P, CHUNK], BF16, tag="gates")
                    func = AF.Tanh if gi == 3 else AF.Sigmoid
                    nc.scalar.activation(g[:, :w2], ps[:, :w2], func,
                                         bias=bias_sb[:, gi * KT:gi * KT + 1],
                                         scale=1.0 / WSCALE)
                    return g

                if store:
                    c_dst = cst[buf][:, t, 0:2 * npar].rearrange(
                        "p (b q) -> p b q", b=2)[:, :, p0:p0 + pw]
                    h_dst = hst[buf][:, t, 0:2 * npar].rearrange(
                        "p (b q) -> p b q", b=2)[:, :, p0:p0 + pw]
                else:
                    c_dst = tpool.tile([P, CHUNK], BF16, tag="tmp",
                                       name="ctmp")[:, :w2].rearrange(
                                           "p (b q) -> p b q", b=2)
                    h_dst = None
                tv = lambda ap: ap.rearrange("p (b q) -> p b q", b=2)
                out_ap = outT_r[:, t, coloff:coloff + 2 * npar].rearrange(
                    "p (b q) -> p b q", b=2)[:, :, p0:p0 + pw]
                blkv = lambda ap: ap.rearrange("p (b q) -> p b q", b=2)
                pc_reps = [(slice(0, w2), blkv,
                            cst[pbuf][:, t, None, p0:p0 + pw].to_broadcast(
                                (P, 2, pw)))]

                def inject(gi):
                    # accumulate the 64-scaled gh into the R half via an
                    # identity matmul from the SBUF copy
                    nc.tensor.matmul(pss[gi][:, pw:w2], id_sb[:], ghs[gi],
                                     start=False, stop=(pw % 512 == 0))

                # software-pipelined gate sequence (i, u, f, o, r) with
                # 2-gate lag between the gh copy and its inject
                acted = {}
                order = [0, 3, 2, 1, 4]
                pss = {}
                ghs = {}
                pipe = []

                def retire(g0):
                    inject(g0)
                    gate_lx(g0, pss[g0])
                    acted[g0] = gate_act(g0, pss[g0])

                for gi in order:
                    if len(pipe) >= 3:
                        retire(pipe.pop(0))
                    ps = pspool.tile([P, CHUNK], F32, tag="ps")
                    pss[gi] = ps
                    gate_h_rx(gi, ps)
                    gh = ghpool.tile([P, PCH], BF16, tag="gh",
                                     name="gh")[:, :pw]
                    nc.vector.tensor_copy(gh, ps[:, 0:pw])
                    ghs[gi] = gh
                    pipe.append(gi)
                retire(pipe.pop(0))  # f
                tc_ = elemwise_early(d, w2, acted[0], acted[3], acted[2],
                                     px, pc_reps, c_dst, tv)
                retire(pipe.pop(0))  # o
                d_ = elemwise_mid(w2, acted[1], tc_, px)
                retire(pipe.pop(0))  # r
                elemwise_late(w2, acted[4], d_, px, tv, h_dst, out_ap)

        for d in range(depth):
            store = d < depth - 1
            buf = d % 2
            n = Ns[d]
            if n <= TPACK_W:
                ftq = fpool.tile([P, KT, CHUNK], FP8, tag="ftq")
                ftb = fpool.tile([P, KT, CHUNK], BF16, tag="ftb")
                nc.sync.dma_start(ftq[:, :, :n],
                                  featsq_r[:, :, off[d]:off[d] + n])
                nc.sync.dma_start(ftb[:, :, :n],
                                  featsb_r[:, :, off[d]:off[d] + n])
                chunk_tpack(d, off[d], 0, 0, n, store, buf, ftq, ftb)
            else:
                npar = n // 2
                for p0 in range(0, npar, PCH):
                    pw = min(PCH, npar - p0)
                    chunk_blk(d, off[d], npar, p0, pw, store, buf)

    nc.compile()
    return nc, C


# ---------------------------------------------------------------- host side

def _col_maps(depth):
    """Per (core, level): global node index for each column. Small levels
    (size <= TPACK_W) use node order (children interleaved: parent of col j
    is col j//2); big levels use block order ([L-children | R-children]:
    parent of col j is col j mod npar). Levels 0-3 are replicated with a
    per-core child-swap so each core's subtree root lands at column 0 of
    level 3."""
    Ns = _level_sizes(depth)
    maps = []
    for i in range(NCORES):
        per_level = []
        cur = np.array([0], dtype=np.int64)
        for d in range(depth):
            if d == 0:
                cur = np.array([0], dtype=np.int64)
            elif d == 4:
                # first private level: children of this core's subtree root
                r = cur[0]
                cur = np.array([2 * r, 2 * r + 1], dtype=np.int64)
            else:
                L, R = 2 * cur, 2 * cur + 1
                if d <= 3:
                    if (i >> (3 - d)) & 1:
                        L, R = R, L
                    cur = np.stack([L, R], axis=1).ravel()
                elif Ns[d] <= TPACK_W:
                    cur = np.stack([L, R], axis=1).ravel()
                else:
                    cur = np.concatenate([L, R])
            per_level.append(((1 << d) - 1) + cur)
        maps.append(per_level)
    return maps


def prep_inputs(features, px_w, px_b, iofux_w, iofux_b, iofuh_w, iofuh_b,
                depth=DEPTH):
    Ns = _level_sizes(depth)
    C = sum(Ns)
    features = np.asarray(features, np.float32)
    wxq = np.ascontiguousarray(
        (np.asarray(iofux_w, np.float32) * WSCALE).T).astype(np_fp8)
    wxpx = np.ascontiguousarray(
        np.asarray(px_w, np.float32).T).astype(np_bf16)
    whq = np.ascontiguousarray(
        (np.asarray(iofuh_w, np.float32) * WHSCALE).T).astype(np_fp8)
    bias_all = np.asarray(iofux_b, np.float32) + np.asarray(iofuh_b, np.float32)
    biasm = np.ascontiguousarray(bias_all.reshape(M_IOFU, P).T)
    pxbm = np.ascontiguousarray(
        np.asarray(px_b, np.float32).reshape(M_PX, P).T)

    maps = _col_maps(depth)
    idm = np.eye(P, dtype=np_bf16)
    in_maps = []
    for i in range(NCORES):
        cols = np.concatenate(maps[i])
        fcore = features[cols, :]                       # [C, 512] f32
        fT = np.ascontiguousarray(fcore.T)              # [512, C]
        in_maps.append({"featsq": fT.astype(np_fp8),
                        "featsb": fT.astype(np_bf16),
                        "wxq": wxq, "wxpx": wxpx, "whq": whq,
                        "biasm": biasm, "pxb": pxbm, "ident": idm})
    return in_maps, maps, C


def assemble_output(results, maps, depth=DEPTH):
    Ns = _level_sizes(depth)
    n_nodes = (1 << depth) - 1
    out = np.empty((n_nodes, H), np.float32)
    offs = np.cumsum([0] + Ns)
    for i in range(NCORES):
        o = results[i]["outT"]                          # [512, C] bf16
        for d in range(depth):
            if d <= 3 and i != 0:
                continue  # replicated levels: take core 0's copy
            cols = maps[i][d]
            out[cols, :] = o[:, offs[d]:offs[d + 1]].T.astype(np.float32)
    return out


_CACHE = {}


def _get_built(depth=DEPTH):
    if depth not in _CACHE:
        _CACHE[depth] = build_nc(depth)
    return _CACHE[depth]


def run_cores(in_maps, depth=DEPTH, trace=False):
    from concourse.bass_utils import run_bass_kernel_spmd
    nc, C = _get_built(depth)
    br = run_bass_kernel_spmd(nc, in_maps, list(range(NCORES)), trace=trace)
    return br


def kernel(features, px_w, px_b, iofux_w, iofux_b, iofuh_w, iofuh_b):
    in_maps, maps, C = prep_inputs(features, px_w, px_b, iofux_w, iofux_b,
                                   iofuh_w, iofuh_b)
    br = run_cores(in_maps)
    return assemble_output(br.results, maps)
